# revision 13
# baseline (speedup 1.0000x reference)
"""Distributed Trainium2 kernel for qk-norm attention (restructured).

Reference computation (B=2, N=2048, C=768, H=12, D=64):
    qkv = x @ W_qkv; q,k,v split per head
    q = LN(q)*scale, k = LN(k)   (LN over head_dim, with gamma/beta)
    out = softmax(q k^T) v ; y = concat_heads(out) @ W_proj + b_proj

Sharding: 24 (batch, head) units -> 8 cores: core c handles batch c//4
and heads 3*(c%4) .. 3*(c%4)+2.  Each core computes a partial
projection y_partial = out_heads @ W_proj[rows]; the host sums the 4
partials per batch and adds b_proj.

Key structural ideas (vs a natural-layout qkv + LN + transpose design):
  - LN centering is linear, so it is folded into the weights host-side:
    W_hat = W_qk @ (I - J/64).  The device never computes means.
  - q/k are computed DIRECTLY in transposed [d, token] layout:
    pair_h = [q_h (partitions 0:64) | k_h (64:128)] = W_pair^T @ x^T,
    so there are no PE transposes at all.
  - Per-token variance: ACT Square of the pair psum -> sq in SBUF, then
    PE ones-matmuls reduce over the partition (d) axis:
      ssq (q rows) in row layout [1, N] for the q-side broadcast,
      ssk (k rows) in column layout [128, 16] for the exp scale.
  - rstd = exp(-0.5*ln(var)) on ACT: ln/exp/square/copy all live in the
    natural_log_exp_and_others table set -> zero table switches.
  - q_hat = q_centered * rstd_q via one gpsimd row-broadcast + one DVE
    multiply; k needs NO elementwise work: 0.125*rstd_k is folded into
    the exp's per-partition scale (softmax rows are k-tokens).
  - exp(0.125*rstd_k*s - 4) directly; qk-normed scores are O(1) so no
    max-subtract is needed.
  - AV with a ones-column in v yields softmax row sums; out^T rows are
    normalized with a reciprocal+broadcast chain overlapped with the
    next head; projection y = oT^T @ W_proj rows with K=192.
"""

import contextlib
import sys

import numpy as np

sys.path.insert(0, "/opt/trn_rl_repo")

import ml_dtypes

import concourse.bass as bass
import concourse.tile as tile
from concourse import bacc, bass_utils, mybir

BF16 = mybir.dt.bfloat16
F32 = mybir.dt.float32

B, N, C = 2, 2048, 768
H, D = 12, 64
HL = 3          # heads per core
P = 128
NT = N // P     # 16 token tiles
KC = C // P     # 6 contraction tiles over C
QC = 512
EPS = 1e-5
EXP_SHIFT = -4.0
SCALE = D ** -0.5  # 0.125


def _build(nc, apply_gb):
    """Emit the per-core program (SPMD: all 8 cores run this graph)."""
    xt_d = nc.dram_tensor("xt", [C, N], BF16, kind="ExternalInput")
    wqk_d = nc.dram_tensor("wqk", [C, HL * P], BF16, kind="ExternalInput")
    wv_d = nc.dram_tensor("wv", [C, HL * D], BF16, kind="ExternalInput")
    wp_d = nc.dram_tensor("wp", [HL * D, C], BF16, kind="ExternalInput")
    gb_d = nc.dram_tensor("gb", [D, 12], F32, kind="ExternalInput")
    out_d = nc.dram_tensor("out", [N, C], BF16, kind="ExternalOutput")

    AF = mybir.ActivationFunctionType

    with tile.TileContext(nc) as tc:
        ctx = contextlib.ExitStack()
        with ctx:
            singles = ctx.enter_context(tc.tile_pool(name="singles", bufs=1))
            persist = ctx.enter_context(tc.tile_pool(name="persist", bufs=1))

            # ---- weights / x^T loads (xt per kc so pair0 can start early;
            # interleaved so the first slices land first) ----
            wqk_sb = persist.tile([P, KC, HL * P], BF16)
            xt_sb = persist.tile([P, KC, N], BF16)
            for kc in range(KC):
                ksl = slice(kc * P, (kc + 1) * P)
                nc.sync.dma_start(out=wqk_sb[:, kc, :], in_=wqk_d.ap()[ksl, :])
                nc.sync.dma_start(out=xt_sb[:, kc, :], in_=xt_d.ap()[ksl, :])
            wv_sb = persist.tile([P, KC, HL * D], BF16)
            for kc in range(KC):
                ksl = slice(kc * P, (kc + 1) * P)
                nc.gpsimd.dma_start(out=wv_sb[:, kc, :], in_=wv_d.ap()[ksl, :])
            wpA = persist.tile([P, C], BF16)
            nc.gpsimd.dma_start(out=wpA, in_=wp_d.ap()[0:P, :])
            wpB = persist.tile([64, C], BF16)
            nc.gpsimd.dma_start(out=wpB, in_=wp_d.ap()[P : P + 64, :])
            gb_sb = singles.tile([64, 12], F32)
            nc.gpsimd.dma_start(out=gb_sb, in_=gb_d.ap())

            # ---- constants ----
            shift_t = singles.tile([P, 1], F32)
            nc.vector.memset(shift_t, EXP_SHIFT)
            zero_t = singles.tile([P, 1], F32)
            nc.vector.memset(zero_t, 0.0)
            epsq_t = singles.tile([P, 1], F32)
            nc.vector.memset(epsq_t, EPS)
            epsk_t = singles.tile([P, 1], F32)
            nc.vector.memset(epsk_t, 64.0 * EPS)
            ones64 = singles.tile([64, 1], BF16)
            nc.vector.memset(ones64, 1.0)
            ones_f1 = singles.tile([1, 1], F32)
            nc.vector.memset(ones_f1, 1.0)
            onesk = singles.tile([P, 1], BF16)
            nc.vector.memset(onesk[0:64, :], 0.0)
            nc.vector.memset(onesk[64:P, :], 1.0)

            # ---- persistent activations ----
            ksb = [
                persist.tile([64, N], BF16, tag=f"ksb{h}", name=f"ksb{h}")
                for h in range(HL)
            ]
            qhat = [
                persist.tile([64, N], BF16, tag=f"qhat{h}", name=f"qhat{h}")
                for h in range(HL)
            ]
            rk_cols = persist.tile([P, HL, NT], F32, tag="rk_cols")
            v_all = persist.tile([P, NT, HL, 66], BF16, tag="v_all")
            nc.gpsimd.memset(v_all[:, :, :, 64:65], 1.0)
            oTA = persist.tile([P, N], BF16, tag="oTA")
            oTB = persist.tile([64, N], BF16, tag="oTB")

            # ============ phase B: qk pairs + stats + v ============
            with tc.tile_pool(name="pairp", bufs=1, space="PSUM") as pairp, \
                 tc.tile_pool(name="ssqp", bufs=1, space="PSUM") as ssqp, \
                 tc.tile_pool(name="vp", bufs=1, space="PSUM") as vp, \
                 tc.tile_pool(name="kstp", bufs=1) as kstp, \
                 tc.tile_pool(name="qrawp", bufs=2) as qrawp, \
                 tc.tile_pool(name="sqp", bufs=2) as sqp, \
                 tc.tile_pool(name="rqp", bufs=2) as rqp, \
                 tc.tile_pool(name="rbp", bufs=1) as rbp:

                vb_count = [0]

                def _emit_v_batch():
                    # two token tiles per batch; v_ps fits one PSUM bank
                    vb = vb_count[0]
                    if vb >= NT // 2:
                        return
                    vb_count[0] += 1
                    v_ps = vp.tile([P, 2, HL * D], F32, tag="v_ps")
                    for m in range(2):
                        mt = 2 * vb + m
                        msl = slice(mt * P, (mt + 1) * P)
                        for kc in range(KC):
                            nc.tensor.matmul(
                                v_ps[:, m, :], xt_sb[:, kc, msl],
                                wv_sb[:, kc, :],
                                start=(kc == 0), stop=(kc == KC - 1),
                            )
                    nc.vector.tensor_copy(
                        v_all[:, 2 * vb : 2 * vb + 2, :, 0:64],
                        v_ps[:].rearrange("p b (h d) -> p b h d", h=HL),
                    )

                for h in range(HL):
                    hsl = slice(h * P, (h + 1) * P)
                    pair = pairp.tile([P, N], F32, tag="pair")
                    if h == 0:
                        # warm fillers gated on each xt slice arrival keep
                        # the PE active through the load phase (HAM ramps
                        # the clock only under sustained activity)
                        nwarm = [5, 5, 4, 3, 2, 1]
                        for kc in range(KC):
                            for j in range(nwarm[kc]):
                                warm = ssqp.tile([1, 2 * QC], F32, tag="ssq")
                                nc.tensor.matmul(
                                    warm[:, 0:QC],
                                    xt_sb[:, kc, 0:1], xt_sb[:, kc, 0:QC],
                                    start=True, stop=True,
                                )
                            for q4 in range(4):
                                nc.tensor.matmul(
                                    pair[:, q4 * QC : (q4 + 1) * QC],
                                    wqk_sb[:, kc, hsl],
                                    xt_sb[:, kc, q4 * QC : (q4 + 1) * QC],
                                    start=(kc == 0), stop=(kc == KC - 1),
                                )
                    else:
                        for kc in range(KC):
                            for q4 in range(4):
                                nc.tensor.matmul(
                                    pair[:, q4 * QC : (q4 + 1) * QC],
                                    wqk_sb[:, kc, hsl],
                                    xt_sb[:, kc, q4 * QC : (q4 + 1) * QC],
                                    start=(kc == 0), stop=(kc == KC - 1),
                                )

                    # squares for the variance reductions (rb chain head)
                    sq = sqp.tile([P, N], BF16, tag="sq")
                    nc.scalar.activation(sq, pair, func=AF.Square, bias=zero_t)
                    # k: psum -> staged copy -> DMA partition-shift to base 0
                    kst = kstp.tile([P, N], BF16, tag="kst")
                    nc.vector.tensor_copy(kst[64:P, :], pair[64:P, :])
                    nc.sync.dma_start(out=ksb[h], in_=kst[64:P, :])
                    # q raw out of psum (frees the pair psum for pair h+1)
                    qraw = qrawp.tile([64, N], BF16, tag="qraw")
                    nc.vector.tensor_copy(qraw, pair[0:64, :])

                    # ssq: row-layout sums of squares over q's 64 partitions
                    ssq = ssqp.tile([1, 2 * QC], F32, tag="ssq")
                    rq = rqp.tile([1, N], F32, tag="rq")
                    for half in range(2):
                        fsl = slice(half * 2 * QC, (half + 1) * 2 * QC)
                        for j in range(2):
                            jsl = slice(j * QC, (j + 1) * QC)
                            nc.tensor.matmul(
                                ssq[:, jsl],
                                ones64, sq[0:64, half * 2 * QC + j * QC :
                                           half * 2 * QC + (j + 1) * QC],
                                start=True, stop=True,
                            )
                        # rstd_q = (ssq/64 + eps)^-1/2; abs_reciprocal_sqrt
                        # shares its table set with square/copy, so phase B
                        # needs no ACT table switches.
                        nc.scalar.activation(
                            rq[:, fsl], ssq, func=AF.Abs_reciprocal_sqrt,
                            bias=epsq_t[0:1, :], scale=1.0 / 64,
                        )
                        if half == 1:
                            _emit_v_batch()
                        if half == 0:
                            ssq = ssqp.tile([1, 2 * QC], F32, tag="ssq")

                    # ssk: row-layout sums of squares over k's partitions
                    # (cheap 512-col matmuls at base 64), then a small DMA
                    # transposes the rstd row into the [128, NT] column form
                    # the exp per-partition scale needs
                    ssk = ssqp.tile([1, 2 * QC], F32, tag="ssq")
                    rk_row = rqp.tile([1, N], F32, tag="rk_row")
                    for half in range(2):
                        fsl = slice(half * 2 * QC, (half + 1) * 2 * QC)
                        for j in range(2):
                            jsl = slice(j * QC, (j + 1) * QC)
                            nc.tensor.matmul(
                                ssk[:, jsl],
                                onesk[64:P, :],
                                sq[64:P, half * 2 * QC + j * QC :
                                   half * 2 * QC + (j + 1) * QC],
                                start=True, stop=True,
                            )
                        # 0.125*rstd_k = (ssk + 64*eps)^-1/2
                        nc.scalar.activation(
                            rk_row[:, fsl], ssk, func=AF.Abs_reciprocal_sqrt,
                            bias=epsk_t[0:1, :], scale=1.0,
                        )
                        if half == 0:
                            ssk = ssqp.tile([1, 2 * QC], F32, tag="ssq")
                    # transpose the rstd row into column form via K=1 PE
                    # transposes (LDWEIGHTS of a single row is ~free)
                    rkT = vp.tile([P, NT], F32, tag="rkT")
                    for kt in range(NT):
                        nc.tensor.transpose(
                            rkT[:, kt : kt + 1],
                            rk_row[:, kt * P : (kt + 1) * P], ones_f1,
                        )
                    nc.scalar.copy(rk_cols[:, h, :], rkT)

                    # broadcast rstd_q across 64 partitions, apply to q
                    rb = rbp.tile([64, N], F32, tag="rb")
                    for half in range(2):
                        fsl = slice(half * N // 2, (half + 1) * N // 2)
                        nc.gpsimd.partition_broadcast(
                            rb[:, fsl], rq[:, fsl], channels=64
                        )
                    nc.vector.tensor_tensor(
                        out=qhat[h], in0=qraw, in1=rb,
                        op=mybir.AluOpType.mult,
                    )
                    if apply_gb:
                        # general gamma/beta: per-partition affines; k also
                        # needs rstd_k applied elementwise (exp scale is
                        # the plain 0.125 const in this mode)
                        nc.vector.tensor_scalar(
                            qhat[h], qhat[h],
                            gb_sb[:, h : h + 1], gb_sb[:, 6 + h : 7 + h],
                            op0=mybir.AluOpType.mult,
                            op1=mybir.AluOpType.add,
                        )
                        ssk2 = ssqp.tile([1, 2 * QC], F32, tag="ssq")
                        rk2 = rqp.tile([1, N], F32, tag="rq")
                        for half in range(2):
                            fsl = slice(half * 2 * QC, (half + 1) * 2 * QC)
                            for j in range(2):
                                jsl = slice(j * QC, (j + 1) * QC)
                                nc.tensor.matmul(
                                    ssk2[:, jsl],
                                    onesk[64:P, :],
                                    sq[64:P, half * 2 * QC + j * QC :
                                       half * 2 * QC + (j + 1) * QC],
                                    start=True, stop=True,
                                )
                            nc.scalar.activation(
                                rk2[:, fsl], ssk2,
                                func=AF.Abs_reciprocal_sqrt,
                                bias=epsq_t[0:1, :], scale=1.0 / 64,
                            )
                            if half == 0:
                                ssk2 = ssqp.tile([1, 2 * QC], F32, tag="ssq")
                        rbk = rbp.tile([64, N], F32, tag="rb")
                        for half in range(2):
                            fsl = slice(half * N // 2, (half + 1) * N // 2)
                            nc.gpsimd.partition_broadcast(
                                rbk[:, fsl], rk2[:, fsl], channels=64
                            )
                        nc.vector.tensor_tensor(
                            out=ksb[h], in0=ksb[h], in1=rbk,
                            op=mybir.AluOpType.mult,
                        )
                        nc.vector.tensor_scalar(
                            ksb[h], ksb[h],
                            gb_sb[:, 3 + h : 4 + h], gb_sb[:, 9 + h : 10 + h],
                            op0=mybir.AluOpType.mult,
                            op1=mybir.AluOpType.add,
                        )

                    # v batches fill the PE while the stat chains drain
                    _emit_v_batch()
                    _emit_v_batch()
                while vb_count[0] < NT // 2:
                    _emit_v_batch()

            # ================= attention =================
            with tc.tile_pool(name="scps", bufs=2, space="PSUM") as psc, \
                 tc.tile_pool(name="avps", bufs=1, space="PSUM") as pav, \
                 tc.tile_pool(name="expsb", bufs=3) as pexp, \
                 tc.tile_pool(name="avfsb", bufs=1) as pavf, \
                 tc.tile_pool(name="rrsb", bufs=2) as prr, \
                 tc.tile_pool(name="rbnsb", bufs=1) as prbn, \
                 tc.tile_pool(name="o1sb", bufs=1) as po1:

                def sc_mms(kT, qT, kt, half):
                    sct = psc.tile([P, 2 * QC], F32, tag="sc")
                    for q2 in range(2):
                        qsl = slice((2 * half + q2) * QC, (2 * half + q2 + 1) * QC)
                        nc.tensor.matmul(
                            sct[:, q2 * QC : (q2 + 1) * QC],
                            kT[:, kt * P : (kt + 1) * P], qT[:, qsl],
                            start=True, stop=True,
                        )
                    return sct

                s0 = s1 = None
                for h in range(HL):
                    kT, qT = ksb[h], qhat[h]
                    escale = rk_cols[:, h, :]
                    av_ps = pav.tile([65, N], F32, tag="av")
                    if s0 is None:
                        s0 = sc_mms(kT, qT, 0, 0)
                        s1 = sc_mms(kT, qT, 0, 1)
                    for kt in range(NT):
                        eT = pexp.tile([P, N], BF16, tag="expT")
                        nc.scalar.activation(
                            eT[:, 0:1024], s0, func=AF.Exp,
                            bias=shift_t,
                            scale=(escale[:, kt : kt + 1] if not apply_gb
                                   else SCALE),
                        )
                        nc.scalar.activation(
                            eT[:, 1024:2048], s1, func=AF.Exp,
                            bias=shift_t,
                            scale=(escale[:, kt : kt + 1] if not apply_gb
                                   else SCALE),
                        )
                        # keep-warm fillers into the dead sc tile
                        nj = 2 if kt % 2 == 0 else 1
                        for j in range(nj):
                            nc.tensor.matmul(
                                s0[:, 0:QC],
                                xt_sb[:, 0, 0:P], xt_sb[:, 0, 0:QC],
                                start=True, stop=True,
                            )
                        if kt < NT - 1:
                            s0n = sc_mms(kT, qT, kt + 1, 0)
                        for qc in (0, 1):
                            nc.tensor.matmul(
                                av_ps[:, qc * QC : (qc + 1) * QC],
                                v_all[:, kt, h, 0:65],
                                eT[:, qc * QC : (qc + 1) * QC],
                                start=(kt == 0), stop=(kt == NT - 1),
                            )
                        if kt < NT - 1:
                            s1n = sc_mms(kT, qT, kt + 1, 1)
                        for qc in (2, 3):
                            nc.tensor.matmul(
                                av_ps[:, qc * QC : (qc + 1) * QC],
                                v_all[:, kt, h, 0:65],
                                eT[:, qc * QC : (qc + 1) * QC],
                                start=(kt == 0), stop=(kt == NT - 1),
                            )
                        if kt < NT - 1:
                            s0, s1 = s0n, s1n
                    # next head's score prologue first so ACT never starves
                    if h + 1 < HL:
                        for j in range(2):
                            nc.tensor.matmul(
                                s1[:, 0:QC],
                                xt_sb[:, 0, 0:P], xt_sb[:, 0, 0:QC],
                                start=True, stop=True,
                            )
                        s0 = sc_mms(ksb[h + 1], qhat[h + 1], 0, 0)
                        s1 = sc_mms(ksb[h + 1], qhat[h + 1], 0, 1)

                    if h < 2:
                        # drain AV psum; normalize overlaps the next head
                        avf = pavf.tile([65, N], F32, tag="avf")
                        nc.vector.tensor_copy(avf, av_ps)
                        # 1/rowsum: [1,2048] -> [4,512] (DMA), recip, back
                        s4 = prr.tile([4, QC], F32, tag="s4")
                        nc.gpsimd.dma_start(out=s4, in_=avf[64:65, :])
                        r4 = prr.tile([4, QC], F32, tag="r4")
                        nc.vector.reciprocal_approx_fast(out=r4, in_=s4)
                        rr = prr.tile([1, N], F32, tag="rr")
                        nc.gpsimd.dma_start(out=rr, in_=r4)
                        rbn = prbn.tile([64, N], F32, tag="rbn")
                        for half in range(2):
                            fsl = slice(half * N // 2, (half + 1) * N // 2)
                            nc.gpsimd.partition_broadcast(
                                rbn[:, fsl], rr[:, fsl], channels=64
                            )
                        if h == 0:
                            nc.vector.tensor_tensor(
                                out=oTA[0:64, :], in0=avf[0:64, :], in1=rbn,
                                op=mybir.AluOpType.mult,
                            )
                        else:
                            # DVE cannot shift partitions; write base-0
                            # tmp then DMA into oTA rows 64-127
                            tmp = po1.tile([64, N], BF16, tag="o1tmp")
                            nc.vector.tensor_tensor(
                                out=tmp, in0=avf[0:64, :], in1=rbn,
                                op=mybir.AluOpType.mult,
                            )
                            nc.sync.dma_start(out=oTA[64:P, :], in_=tmp)
                    else:
                        # h2 gates the projection: process in q-halves so
                        # proj mts 0-7 start while half 1 is in flight
                        avf = pavf.tile([65, N], F32, tag="avf")
                        for half in range(2):
                            fsl = slice(half * 1024, (half + 1) * 1024)
                            nc.vector.tensor_copy(avf[:, fsl], av_ps[:, fsl])
                            s2 = prr.tile([2, QC], F32, tag="s4")
                            nc.gpsimd.dma_start(out=s2, in_=avf[64:65, fsl])
                            r2 = prr.tile([2, QC], F32, tag="r4")
                            nc.vector.reciprocal_approx_fast(out=r2, in_=s2)
                            rr2 = prr.tile([1, N], F32, tag="rr")
                            nc.gpsimd.dma_start(out=rr2[:, fsl], in_=r2)
                            rbn2 = prbn.tile([64, N], F32, tag="rbn")
                            nc.gpsimd.partition_broadcast(
                                rbn2[:, fsl], rr2[:, fsl], channels=64
                            )
                            nc.vector.tensor_tensor(
                                out=oTB[:, fsl], in0=avf[0:64, fsl],
                                in1=rbn2[:, fsl],
                                op=mybir.AluOpType.mult,
                            )

            # ================= projection =================
            with tc.tile_pool(name="pjps", bufs=3, space="PSUM") as ppj, \
                 tc.tile_pool(name="pjw", bufs=2, space="PSUM") as ppw, \
                 tc.tile_pool(name="ysb", bufs=3) as py:
                for mt in range(NT):
                    msl = slice(mt * P, (mt + 1) * P)
                    y_ps = ppj.tile([P, C], F32, tag="y")
                    # warm filler keeps the HAM activity monitor fed so the
                    # PE stays at full clock through the projection
                    warmp = ppw.tile([P, P], F32, tag="warmp")
                    nc.tensor.matmul(
                        warmp, xt_sb[:, 0, 0:P], xt_sb[:, 0, 0:P],
                        start=True, stop=True,
                    )
                    for n0, n1 in [(0, 512), (512, 768)]:
                        nc.tensor.matmul(
                            y_ps[:, n0:n1], oTA[:, msl], wpA[:, n0:n1],
                            start=True, stop=False,
                        )
                        nc.tensor.matmul(
                            y_ps[:, n0:n1], oTB[:, msl], wpB[:, n0:n1],
                            start=False, stop=True,
                        )
                    y_out = py.tile([P, C], BF16, tag="y_out")
                    # split the drain across both copy engines
                    nc.vector.tensor_copy(y_out[:, 0:384], y_ps[:, 0:384])
                    nc.scalar.copy(y_out[:, 384:768], y_ps[:, 384:768])
                    nc.sync.dma_start(out=out_d.ap()[msl, :], in_=y_out)

    nc.compile()
    return nc


_CACHED = {}


def _get_nc(apply_gb):
    key = ("nc", apply_gb)
    if key not in _CACHED:
        nc = bacc.Bacc("TRN2", target_bir_lowering=False, debug=False)
        _CACHED[key] = _build(nc, apply_gb)
    return _CACHED[key]


def _make_in_maps(inputs):
    x = np.asarray(inputs["x"], np.float32)
    wqkv = np.asarray(inputs["W_qkv"], np.float32)
    wproj = np.asarray(inputs["W_proj"], np.float32)
    qg = np.asarray(inputs["q_gamma"], np.float32)
    qb = np.asarray(inputs["q_beta"], np.float32)
    kg = np.asarray(inputs["k_gamma"], np.float32)
    kb = np.asarray(inputs["k_beta"], np.float32)

    bf = ml_dtypes.bfloat16
    w3 = wqkv.reshape(C, 3, H, D)
    cmat = np.eye(D, dtype=np.float32) - np.full((D, D), 1.0 / D, np.float32)
    in_maps = []
    for c in range(8):
        b = c // 4
        h0 = (c % 4) * HL
        cols = []
        for hh in range(HL):
            cols.append(w3[:, 0, h0 + hh, :] @ cmat)  # centered Wq
            cols.append(w3[:, 1, h0 + hh, :] @ cmat)  # centered Wk
        wqk = np.concatenate(cols, axis=1)  # [C, 384]
        wv = np.ascontiguousarray(
            w3[:, 2, h0 : h0 + HL, :].reshape(C, HL * D)
        )
        gbm = np.zeros((D, 12), np.float32)
        gbm[:, 0:3] = qg[:, None]
        gbm[:, 3:6] = kg[:, None]
        gbm[:, 6:9] = qb[:, None]
        gbm[:, 9:12] = kb[:, None]
        in_maps.append(
            {
                "xt": np.ascontiguousarray(x[b].T).astype(bf),
                "wqk": np.ascontiguousarray(wqk).astype(bf),
                "wv": wv.astype(bf),
                "wp": np.ascontiguousarray(
                    wproj[h0 * D : (h0 + HL) * D, :]
                ).astype(bf),
                "gb": gbm,
            }
        )
    return in_maps


def _gather(inputs, results):
    bproj = np.asarray(inputs["b_proj"], np.float32)
    y = np.zeros((B, N, C), np.float32)
    for c in range(8):
        y[c // 4] += np.asarray(results[c]["out"], dtype=np.float32)
    y += bproj
    return y


def _install_profile_hook():
    """The agent image's antenv lacks axon_hooks; synthesize it so
    run_bass_kernel_spmd(trace=True) can NTFF-profile via ctypes."""
    import types

    if "antenv.axon_hooks" in sys.modules:
        return
    try:
        from trn_agent_boot.trn_boot import _ntff_profile_via_ctypes

        hook = _ntff_profile_via_ctypes("/opt/axon/libaxon_pjrt.so")
    except Exception:
        hook = None
    mod = types.ModuleType("antenv.axon_hooks")
    mod.get_axon_ntff_profile_hook = lambda: hook
    mod.set_axon_ntff_profile_hook = lambda h: None
    sys.modules["antenv.axon_hooks"] = mod
    # no S3 in this container: keep artifacts local
    bass_utils.upload_artifacts = lambda tmpdir: tmpdir


def _kernel_impl(inputs, trace=False, tmpdir=None):
    apply_gb = not (
        np.all(np.asarray(inputs["q_gamma"]) == 1.0)
        and np.all(np.asarray(inputs["k_gamma"]) == 1.0)
        and np.all(np.asarray(inputs["q_beta"]) == 0.0)
        and np.all(np.asarray(inputs["k_beta"]) == 0.0)
    )
    nc = _get_nc(apply_gb)
    in_maps = _make_in_maps(inputs)
    if trace:
        _install_profile_hook()
    res = bass_utils.run_bass_kernel_spmd(
        nc, in_maps, core_ids=list(range(8)), trace=trace, tmpdir=tmpdir
    )
    out = _gather(inputs, res.results)
    return out, res


def kernel(**inputs):
    out, _ = _kernel_impl(inputs)
    return out


def kernel_with_profile(**inputs):
    out, res = _kernel_impl(inputs, trace=True)
    return out, res


# revision 16
# speedup vs baseline: 1.2010x; 1.2010x over previous
"""Distributed Trainium2 kernel for qk-norm attention (restructured).

Reference computation (B=2, N=2048, C=768, H=12, D=64):
    qkv = x @ W_qkv; q,k,v split per head
    q = LN(q)*scale, k = LN(k)   (LN over head_dim, with gamma/beta)
    out = softmax(q k^T) v ; y = concat_heads(out) @ W_proj + b_proj

Sharding: 24 (batch, head) units -> 8 cores: core c handles batch c//4
and heads 3*(c%4) .. 3*(c%4)+2.  Each core computes a partial
projection y_partial = out_heads @ W_proj[rows]; the host sums the 4
partials per batch and adds b_proj.

Key structural ideas (vs a natural-layout qkv + LN + transpose design):
  - LN centering is linear, so it is folded into the weights host-side:
    W_hat = W_qk @ (I - J/64).  The device never computes means.
  - q/k are computed DIRECTLY in transposed [d, token] layout:
    pair_h = [q_h (partitions 0:64) | k_h (64:128)] = W_pair^T @ x^T,
    so there are no PE transposes at all.
  - Per-token variance: ACT Square of the pair psum -> sq in SBUF, then
    PE ones-matmuls reduce over the partition (d) axis:
      ssq (q rows) in row layout [1, N] for the q-side broadcast,
      ssk (k rows) in column layout [128, 16] for the exp scale.
  - rstd = exp(-0.5*ln(var)) on ACT: ln/exp/square/copy all live in the
    natural_log_exp_and_others table set -> zero table switches.
  - q_hat = q_centered * rstd_q via one gpsimd row-broadcast + one DVE
    multiply; k needs NO elementwise work: 0.125*rstd_k is folded into
    the exp's per-partition scale (softmax rows are k-tokens).
  - exp(0.125*rstd_k*s - 4) directly; qk-normed scores are O(1) so no
    max-subtract is needed.
  - AV with a ones-column in v yields softmax row sums; out^T rows are
    normalized with a reciprocal+broadcast chain overlapped with the
    next head; projection y = oT^T @ W_proj rows with K=192.
"""

import contextlib
import sys

import numpy as np

sys.path.insert(0, "/opt/trn_rl_repo")

import ml_dtypes

import concourse.bass as bass
import concourse.tile as tile
from concourse import bacc, bass_utils, mybir
from concourse.masks import make_identity

BF16 = mybir.dt.bfloat16
F32 = mybir.dt.float32

B, N, C = 2, 2048, 768
H, D = 12, 64
HL = 3          # heads per core
P = 128
NT = N // P     # 16 token tiles
KC = C // P     # 6 contraction tiles over C
QC = 512
EPS = 1e-5
EXP_SHIFT = -4.0
SCALE = D ** -0.5  # 0.125


def _build(nc, apply_gb):
    """Emit the per-core program (SPMD: all 8 cores run this graph)."""
    xt_d = nc.dram_tensor("xt", [C, N], BF16, kind="ExternalInput")
    wqk_d = nc.dram_tensor("wqk", [C, HL * P], BF16, kind="ExternalInput")
    wv_d = nc.dram_tensor("wv", [C, HL * D], BF16, kind="ExternalInput")
    wp_d = nc.dram_tensor("wp", [HL * D, C], BF16, kind="ExternalInput")
    gb_d = nc.dram_tensor("gb", [D, 12], F32, kind="ExternalInput")
    out_d = nc.dram_tensor("out", [N, C], BF16, kind="ExternalOutput")

    AF = mybir.ActivationFunctionType

    with tile.TileContext(nc) as tc:
        ctx = contextlib.ExitStack()
        with ctx:
            singles = ctx.enter_context(tc.tile_pool(name="singles", bufs=1))
            persist = ctx.enter_context(tc.tile_pool(name="persist", bufs=1))

            # ---- weights / x^T loads (xt per kc so pair0 can start early;
            # interleaved so the first slices land first) ----
            wqk_sb = persist.tile([P, KC, HL * P], BF16)
            xt_sb = persist.tile([P, KC, N], BF16)
            for kc in range(KC):
                ksl = slice(kc * P, (kc + 1) * P)
                nc.sync.dma_start(out=wqk_sb[:, kc, :], in_=wqk_d.ap()[ksl, :])
                nc.sync.dma_start(out=xt_sb[:, kc, :], in_=xt_d.ap()[ksl, :])
            wv_sb = persist.tile([P, KC, HL * D], BF16)
            for kc in range(KC):
                ksl = slice(kc * P, (kc + 1) * P)
                nc.gpsimd.dma_start(out=wv_sb[:, kc, :], in_=wv_d.ap()[ksl, :])
            wpA = persist.tile([P, C], BF16)
            nc.gpsimd.dma_start(out=wpA, in_=wp_d.ap()[0:P, :])
            wpB = persist.tile([64, C], BF16)
            nc.gpsimd.dma_start(out=wpB, in_=wp_d.ap()[P : P + 64, :])
            gb_sb = singles.tile([64, 12], F32)
            nc.gpsimd.dma_start(out=gb_sb, in_=gb_d.ap())

            # ---- constants ----
            shift_t = singles.tile([P, 1], F32)
            nc.vector.memset(shift_t, EXP_SHIFT)
            zero_t = singles.tile([P, 1], F32)
            nc.vector.memset(zero_t, 0.0)
            epsq_t = singles.tile([P, 1], F32)
            nc.vector.memset(epsq_t, EPS)
            epsk_t = singles.tile([P, 1], F32)
            nc.vector.memset(epsk_t, 64.0 * EPS)
            ones64 = singles.tile([64, 1], BF16)
            nc.vector.memset(ones64, 1.0)
            ident16 = singles.tile([NT, NT], F32)
            make_identity(nc, ident16)
            onesk = singles.tile([P, 1], BF16)
            nc.vector.memset(onesk[0:64, :], 0.0)
            nc.vector.memset(onesk[64:P, :], 1.0)

            # ---- persistent activations ----
            ksb = [
                persist.tile([64, N], BF16, tag=f"ksb{h}", name=f"ksb{h}")
                for h in range(HL)
            ]
            qhat = [
                persist.tile([64, N], BF16, tag=f"qhat{h}", name=f"qhat{h}")
                for h in range(HL)
            ]
            rk_cols = persist.tile([P, HL, NT], F32, tag="rk_cols")
            v_all = persist.tile([P, NT, HL, 66], BF16, tag="v_all")
            nc.gpsimd.memset(v_all[:, :, :, 64:65], 1.0)
            oTA = persist.tile([P, N], BF16, tag="oTA")
            oTB = persist.tile([64, N], BF16, tag="oTB")

            # ============ phase B: qk pairs + stats + v ============
            with tc.tile_pool(name="pairp", bufs=1, space="PSUM") as pairp, \
                 tc.tile_pool(name="ssqp", bufs=1, space="PSUM") as ssqp, \
                 tc.tile_pool(name="vp", bufs=1, space="PSUM") as vp, \
                 tc.tile_pool(name="kstp", bufs=1) as kstp, \
                 tc.tile_pool(name="qrawp", bufs=2) as qrawp, \
                 tc.tile_pool(name="sqp", bufs=2) as sqp, \
                 tc.tile_pool(name="rqp", bufs=2) as rqp, \
                 tc.tile_pool(name="rbp", bufs=1) as rbp:

                vb_count = [0]

                def _emit_v_batch():
                    # two token tiles per batch; v_ps fits one PSUM bank
                    vb = vb_count[0]
                    if vb >= NT // 2:
                        return
                    vb_count[0] += 1
                    v_ps = vp.tile([P, 2, HL * D], F32, tag="v_ps")
                    for m in range(2):
                        mt = 2 * vb + m
                        msl = slice(mt * P, (mt + 1) * P)
                        for kc in range(KC):
                            nc.tensor.matmul(
                                v_ps[:, m, :], xt_sb[:, kc, msl],
                                wv_sb[:, kc, :],
                                start=(kc == 0), stop=(kc == KC - 1),
                            )
                    nc.vector.tensor_copy(
                        v_all[:, 2 * vb : 2 * vb + 2, :, 0:64],
                        v_ps[:].rearrange("p b (h d) -> p b h d", h=HL),
                    )

                rk16s = []
                for h in range(HL):
                    hsl = slice(h * P, (h + 1) * P)
                    pair = pairp.tile([P, N], F32, tag="pair")
                    if h == 0:
                        # warm fillers gated on each xt slice arrival keep
                        # the PE active through the load phase (HAM ramps
                        # the clock only under sustained activity)
                        nwarm = [5, 5, 4, 3, 2, 1]
                        for kc in range(KC):
                            for j in range(nwarm[kc]):
                                warm = ssqp.tile([1, QC], F32, tag="ssq")
                                nc.tensor.matmul(
                                    warm,
                                    xt_sb[:, kc, 0:1], xt_sb[:, kc, 0:QC],
                                    start=True, stop=True,
                                )
                            for q4 in range(4):
                                nc.tensor.matmul(
                                    pair[:, q4 * QC : (q4 + 1) * QC],
                                    wqk_sb[:, kc, hsl],
                                    xt_sb[:, kc, q4 * QC : (q4 + 1) * QC],
                                    start=(kc == 0), stop=(kc == KC - 1),
                                )
                    else:
                        for kc in range(KC):
                            for q4 in range(4):
                                nc.tensor.matmul(
                                    pair[:, q4 * QC : (q4 + 1) * QC],
                                    wqk_sb[:, kc, hsl],
                                    xt_sb[:, kc, q4 * QC : (q4 + 1) * QC],
                                    start=(kc == 0), stop=(kc == KC - 1),
                                )

                    # squares for the variance reductions (rb chain head)
                    sq = sqp.tile([P, N], BF16, tag="sq")
                    nc.scalar.activation(sq, pair, func=AF.Square, bias=zero_t)
                    # k: psum -> staged copy -> DMA partition-shift to base 0
                    kst = kstp.tile([P, N], BF16, tag="kst")
                    nc.vector.tensor_copy(kst[64:P, :], pair[64:P, :])
                    nc.sync.dma_start(out=ksb[h], in_=kst[64:P, :])
                    # q raw out of psum (frees the pair psum for pair h+1)
                    qraw = qrawp.tile([64, N], BF16, tag="qraw")
                    nc.vector.tensor_copy(qraw, pair[0:64, :])

                    # v batches fill the PE while ACT computes the squares
                    _emit_v_batch()
                    _emit_v_batch()

                    # ssq/ssk: row-layout sums of squares over the 64 q (k)
                    # partitions; rstd = (ss*scale + eps)^-1/2 on ACT.
                    # abs_reciprocal_sqrt shares its table set with
                    # square/copy, so phase B needs no ACT table switches.
                    rq = rqp.tile([1, N], F32, tag="rq")
                    rk_row = rqp.tile([1, N], F32, tag="rk_row")
                    for qt in range(4):
                        fsl = slice(qt * QC, (qt + 1) * QC)
                        ssq = ssqp.tile([1, QC], F32, tag="ssq")
                        ssk = ssqp.tile([1, QC], F32, tag="ssk")
                        nc.tensor.matmul(
                            ssq, ones64, sq[0:64, fsl],
                            start=True, stop=True,
                        )
                        nc.tensor.matmul(
                            ssk, onesk[64:P, :], sq[64:P, fsl],
                            start=True, stop=True,
                        )
                        nc.scalar.activation(
                            rq[:, fsl], ssq, func=AF.Abs_reciprocal_sqrt,
                            bias=epsq_t[0:1, :], scale=1.0 / 64,
                        )
                        # 0.125*rstd_k = (ssk + 64*eps)^-1/2
                        nc.scalar.activation(
                            rk_row[:, fsl], ssk, func=AF.Abs_reciprocal_sqrt,
                            bias=epsk_t[0:1, :], scale=1.0,
                        )
                    # reshape the rstd_k row to [16, 128]; a single PE
                    # transpose at the end of phase B makes it [128, 16]
                    rk16 = persist.tile(
                        [NT, P], F32, tag=f"rk16_{h}", name=f"rk16_{h}"
                    )
                    nc.gpsimd.dma_start(out=rk16, in_=rk_row)
                    rk16s.append(rk16)

                    # broadcast rstd_q across 64 partitions, apply to q
                    rb = rbp.tile([64, N], F32, tag="rb")
                    for half in range(2):
                        fsl = slice(half * N // 2, (half + 1) * N // 2)
                        nc.gpsimd.partition_broadcast(
                            rb[:, fsl], rq[:, fsl], channels=64
                        )
                    nc.vector.tensor_tensor(
                        out=qhat[h], in0=qraw, in1=rb,
                        op=mybir.AluOpType.mult,
                    )
                    if apply_gb:
                        # general gamma/beta: per-partition affines; k also
                        # needs rstd_k applied elementwise (exp scale is
                        # the plain 0.125 const in this mode)
                        nc.vector.tensor_scalar(
                            qhat[h], qhat[h],
                            gb_sb[:, h : h + 1], gb_sb[:, 6 + h : 7 + h],
                            op0=mybir.AluOpType.mult,
                            op1=mybir.AluOpType.add,
                        )
                        rk2 = rqp.tile([1, N], F32, tag="rq")
                        for qt in range(4):
                            fsl = slice(qt * QC, (qt + 1) * QC)
                            ssk2 = ssqp.tile([1, QC], F32, tag="ssk")
                            nc.tensor.matmul(
                                ssk2, onesk[64:P, :], sq[64:P, fsl],
                                start=True, stop=True,
                            )
                            nc.scalar.activation(
                                rk2[:, fsl], ssk2,
                                func=AF.Abs_reciprocal_sqrt,
                                bias=epsq_t[0:1, :], scale=1.0 / 64,
                            )
                        rbk = rbp.tile([64, N], F32, tag="rb")
                        for half in range(2):
                            fsl = slice(half * N // 2, (half + 1) * N // 2)
                            nc.gpsimd.partition_broadcast(
                                rbk[:, fsl], rk2[:, fsl], channels=64
                            )
                        nc.vector.tensor_tensor(
                            out=ksb[h], in0=ksb[h], in1=rbk,
                            op=mybir.AluOpType.mult,
                        )
                        nc.vector.tensor_scalar(
                            ksb[h], ksb[h],
                            gb_sb[:, 3 + h : 4 + h], gb_sb[:, 9 + h : 10 + h],
                            op0=mybir.AluOpType.mult,
                            op1=mybir.AluOpType.add,
                        )

                    # v batches fill the PE while the stat chains drain
                    _emit_v_batch()
                    _emit_v_batch()
                while vb_count[0] < NT // 2:
                    _emit_v_batch()
                # one PE transpose per head turns the [16, 128] rstd_k
                # reshape into the [128, 16] exp-scale column form
                for h in range(HL):
                    rkT = vp.tile([P, NT], F32, tag="rkT")
                    nc.tensor.transpose(rkT, rk16s[h], ident16)
                    nc.scalar.copy(rk_cols[:, h, :], rkT)

            # ================= attention =================
            with tc.tile_pool(name="scps", bufs=2, space="PSUM") as psc, \
                 tc.tile_pool(name="avps", bufs=1, space="PSUM") as pav, \
                 tc.tile_pool(name="expsb", bufs=3) as pexp, \
                 tc.tile_pool(name="avfsb", bufs=1) as pavf, \
                 tc.tile_pool(name="rrsb", bufs=2) as prr, \
                 tc.tile_pool(name="rbnsb", bufs=1) as prbn, \
                 tc.tile_pool(name="o1sb", bufs=1) as po1:

                def sc_mms(kT, qT, kt, half):
                    sct = psc.tile([P, 2 * QC], F32, tag="sc")
                    for q2 in range(2):
                        qsl = slice((2 * half + q2) * QC, (2 * half + q2 + 1) * QC)
                        nc.tensor.matmul(
                            sct[:, q2 * QC : (q2 + 1) * QC],
                            kT[:, kt * P : (kt + 1) * P], qT[:, qsl],
                            start=True, stop=True,
                        )
                    return sct

                s0 = s1 = None
                for h in range(HL):
                    kT, qT = ksb[h], qhat[h]
                    escale = rk_cols[:, h, :]
                    av_ps = pav.tile([65, N], F32, tag="av")
                    if s0 is None:
                        s0 = sc_mms(kT, qT, 0, 0)
                        s1 = sc_mms(kT, qT, 0, 1)
                    for kt in range(NT):
                        eT = pexp.tile([P, N], BF16, tag="expT")
                        nc.scalar.activation(
                            eT[:, 0:1024], s0, func=AF.Exp,
                            bias=shift_t,
                            scale=(escale[:, kt : kt + 1] if not apply_gb
                                   else SCALE),
                        )
                        nc.scalar.activation(
                            eT[:, 1024:2048], s1, func=AF.Exp,
                            bias=shift_t,
                            scale=(escale[:, kt : kt + 1] if not apply_gb
                                   else SCALE),
                        )
                        # keep-warm fillers into the dead sc tile
                        nj = 2 if kt % 2 == 0 else 1
                        for j in range(nj):
                            nc.tensor.matmul(
                                s0[:, 0:QC],
                                xt_sb[:, 0, 0:P], xt_sb[:, 0, 0:QC],
                                start=True, stop=True,
                            )
                        if kt < NT - 1:
                            s0n = sc_mms(kT, qT, kt + 1, 0)
                        for qc in (0, 1):
                            nc.tensor.matmul(
                                av_ps[:, qc * QC : (qc + 1) * QC],
                                v_all[:, kt, h, 0:65],
                                eT[:, qc * QC : (qc + 1) * QC],
                                start=(kt == 0), stop=(kt == NT - 1),
                            )
                        if kt < NT - 1:
                            s1n = sc_mms(kT, qT, kt + 1, 1)
                        for qc in (2, 3):
                            nc.tensor.matmul(
                                av_ps[:, qc * QC : (qc + 1) * QC],
                                v_all[:, kt, h, 0:65],
                                eT[:, qc * QC : (qc + 1) * QC],
                                start=(kt == 0), stop=(kt == NT - 1),
                            )
                        if kt < NT - 1:
                            s0, s1 = s0n, s1n
                    # next head's score prologue first so ACT never starves
                    if h + 1 < HL:
                        for j in range(2):
                            nc.tensor.matmul(
                                s1[:, 0:QC],
                                xt_sb[:, 0, 0:P], xt_sb[:, 0, 0:QC],
                                start=True, stop=True,
                            )
                        s0 = sc_mms(ksb[h + 1], qhat[h + 1], 0, 0)
                        s1 = sc_mms(ksb[h + 1], qhat[h + 1], 0, 1)

                    if h < 2:
                        # drain AV psum; normalize overlaps the next head
                        avf = pavf.tile([65, N], F32, tag="avf")
                        nc.vector.tensor_copy(avf, av_ps)
                        # 1/rowsum: [1,2048] -> [4,512] (DMA), recip, back
                        s4 = prr.tile([4, QC], F32, tag="s4")
                        nc.gpsimd.dma_start(out=s4, in_=avf[64:65, :])
                        r4 = prr.tile([4, QC], F32, tag="r4")
                        nc.vector.reciprocal_approx_fast(out=r4, in_=s4)
                        rr = prr.tile([1, N], F32, tag="rr")
                        nc.gpsimd.dma_start(out=rr, in_=r4)
                        rbn = prbn.tile([64, N], F32, tag="rbn")
                        for half in range(2):
                            fsl = slice(half * N // 2, (half + 1) * N // 2)
                            nc.gpsimd.partition_broadcast(
                                rbn[:, fsl], rr[:, fsl], channels=64
                            )
                        if h == 0:
                            nc.vector.tensor_tensor(
                                out=oTA[0:64, :], in0=avf[0:64, :], in1=rbn,
                                op=mybir.AluOpType.mult,
                            )
                        else:
                            # DVE cannot shift partitions; write base-0
                            # tmp then DMA into oTA rows 64-127
                            tmp = po1.tile([64, N], BF16, tag="o1tmp")
                            nc.vector.tensor_tensor(
                                out=tmp, in0=avf[0:64, :], in1=rbn,
                                op=mybir.AluOpType.mult,
                            )
                            nc.sync.dma_start(out=oTA[64:P, :], in_=tmp)
                    else:
                        # h2 gates the projection: process in q-halves so
                        # proj mts 0-7 start while half 1 is in flight
                        avf = pavf.tile([65, N], F32, tag="avf")
                        for half in range(2):
                            fsl = slice(half * 1024, (half + 1) * 1024)
                            nc.vector.tensor_copy(avf[:, fsl], av_ps[:, fsl])
                            s2 = prr.tile([2, QC], F32, tag="s4")
                            nc.gpsimd.dma_start(out=s2, in_=avf[64:65, fsl])
                            r2 = prr.tile([2, QC], F32, tag="r4")
                            nc.vector.reciprocal_approx_fast(out=r2, in_=s2)
                            rr2 = prr.tile([1, N], F32, tag="rr")
                            nc.gpsimd.dma_start(out=rr2[:, fsl], in_=r2)
                            rbn2 = prbn.tile([64, N], F32, tag="rbn")
                            nc.gpsimd.partition_broadcast(
                                rbn2[:, fsl], rr2[:, fsl], channels=64
                            )
                            nc.vector.tensor_tensor(
                                out=oTB[:, fsl], in0=avf[0:64, fsl],
                                in1=rbn2[:, fsl],
                                op=mybir.AluOpType.mult,
                            )

            # ================= projection =================
            with tc.tile_pool(name="pjps", bufs=3, space="PSUM") as ppj, \
                 tc.tile_pool(name="pjw", bufs=2, space="PSUM") as ppw, \
                 tc.tile_pool(name="ysb", bufs=3) as py:
                for mt in range(NT):
                    msl = slice(mt * P, (mt + 1) * P)
                    y_ps = ppj.tile([P, C], F32, tag="y")
                    # warm filler keeps the HAM activity monitor fed so the
                    # PE stays at full clock through the projection
                    warmp = ppw.tile([P, P], F32, tag="warmp")
                    nc.tensor.matmul(
                        warmp, xt_sb[:, 0, 0:P], xt_sb[:, 0, 0:P],
                        start=True, stop=True,
                    )
                    for n0, n1 in [(0, 512), (512, 768)]:
                        nc.tensor.matmul(
                            y_ps[:, n0:n1], oTA[:, msl], wpA[:, n0:n1],
                            start=True, stop=False,
                        )
                        nc.tensor.matmul(
                            y_ps[:, n0:n1], oTB[:, msl], wpB[:, n0:n1],
                            start=False, stop=True,
                        )
                    y_out = py.tile([P, C], BF16, tag="y_out")
                    # split the drain across both copy engines
                    nc.vector.tensor_copy(y_out[:, 0:384], y_ps[:, 0:384])
                    nc.scalar.copy(y_out[:, 384:768], y_ps[:, 384:768])
                    nc.sync.dma_start(out=out_d.ap()[msl, :], in_=y_out)

    nc.compile()
    return nc


_CACHED = {}


def _get_nc(apply_gb):
    key = ("nc", apply_gb)
    if key not in _CACHED:
        nc = bacc.Bacc("TRN2", target_bir_lowering=False, debug=False)
        _CACHED[key] = _build(nc, apply_gb)
    return _CACHED[key]


def _make_in_maps(inputs):
    x = np.asarray(inputs["x"], np.float32)
    wqkv = np.asarray(inputs["W_qkv"], np.float32)
    wproj = np.asarray(inputs["W_proj"], np.float32)
    qg = np.asarray(inputs["q_gamma"], np.float32)
    qb = np.asarray(inputs["q_beta"], np.float32)
    kg = np.asarray(inputs["k_gamma"], np.float32)
    kb = np.asarray(inputs["k_beta"], np.float32)

    bf = ml_dtypes.bfloat16
    w3 = wqkv.reshape(C, 3, H, D)
    cmat = np.eye(D, dtype=np.float32) - np.full((D, D), 1.0 / D, np.float32)
    in_maps = []
    for c in range(8):
        b = c // 4
        h0 = (c % 4) * HL
        cols = []
        for hh in range(HL):
            cols.append(w3[:, 0, h0 + hh, :] @ cmat)  # centered Wq
            cols.append(w3[:, 1, h0 + hh, :] @ cmat)  # centered Wk
        wqk = np.concatenate(cols, axis=1)  # [C, 384]
        wv = np.ascontiguousarray(
            w3[:, 2, h0 : h0 + HL, :].reshape(C, HL * D)
        )
        gbm = np.zeros((D, 12), np.float32)
        gbm[:, 0:3] = qg[:, None]
        gbm[:, 3:6] = kg[:, None]
        gbm[:, 6:9] = qb[:, None]
        gbm[:, 9:12] = kb[:, None]
        in_maps.append(
            {
                "xt": np.ascontiguousarray(x[b].T).astype(bf),
                "wqk": np.ascontiguousarray(wqk).astype(bf),
                "wv": wv.astype(bf),
                "wp": np.ascontiguousarray(
                    wproj[h0 * D : (h0 + HL) * D, :]
                ).astype(bf),
                "gb": gbm,
            }
        )
    return in_maps


def _gather(inputs, results):
    bproj = np.asarray(inputs["b_proj"], np.float32)
    y = np.zeros((B, N, C), np.float32)
    for c in range(8):
        y[c // 4] += np.asarray(results[c]["out"], dtype=np.float32)
    y += bproj
    return y


def _install_profile_hook():
    """The agent image's antenv lacks axon_hooks; synthesize it so
    run_bass_kernel_spmd(trace=True) can NTFF-profile via ctypes."""
    import types

    if "antenv.axon_hooks" in sys.modules:
        return
    try:
        from trn_agent_boot.trn_boot import _ntff_profile_via_ctypes

        hook = _ntff_profile_via_ctypes("/opt/axon/libaxon_pjrt.so")
    except Exception:
        hook = None
    mod = types.ModuleType("antenv.axon_hooks")
    mod.get_axon_ntff_profile_hook = lambda: hook
    mod.set_axon_ntff_profile_hook = lambda h: None
    sys.modules["antenv.axon_hooks"] = mod
    # no S3 in this container: keep artifacts local
    bass_utils.upload_artifacts = lambda tmpdir: tmpdir


def _kernel_impl(inputs, trace=False, tmpdir=None):
    apply_gb = not (
        np.all(np.asarray(inputs["q_gamma"]) == 1.0)
        and np.all(np.asarray(inputs["k_gamma"]) == 1.0)
        and np.all(np.asarray(inputs["q_beta"]) == 0.0)
        and np.all(np.asarray(inputs["k_beta"]) == 0.0)
    )
    nc = _get_nc(apply_gb)
    in_maps = _make_in_maps(inputs)
    if trace:
        _install_profile_hook()
    res = bass_utils.run_bass_kernel_spmd(
        nc, in_maps, core_ids=list(range(8)), trace=trace, tmpdir=tmpdir
    )
    out = _gather(inputs, res.results)
    return out, res


def kernel(**inputs):
    out, _ = _kernel_impl(inputs)
    return out


def kernel_with_profile(**inputs):
    out, res = _kernel_impl(inputs, trace=True)
    return out, res


# revision 17
# speedup vs baseline: 1.2674x; 1.0553x over previous
"""Distributed Trainium2 kernel for qk-norm attention (restructured).

Reference computation (B=2, N=2048, C=768, H=12, D=64):
    qkv = x @ W_qkv; q,k,v split per head
    q = LN(q)*scale, k = LN(k)   (LN over head_dim, with gamma/beta)
    out = softmax(q k^T) v ; y = concat_heads(out) @ W_proj + b_proj

Sharding: 24 (batch, head) units -> 8 cores: core c handles batch c//4
and heads 3*(c%4) .. 3*(c%4)+2.  Each core computes a partial
projection y_partial = out_heads @ W_proj[rows]; the host sums the 4
partials per batch and adds b_proj.

Key structural ideas (vs a natural-layout qkv + LN + transpose design):
  - LN centering is linear, so it is folded into the weights host-side:
    W_hat = W_qk @ (I - J/64).  The device never computes means.
  - q/k are computed DIRECTLY in transposed [d, token] layout:
    pair_h = [q_h (partitions 0:64) | k_h (64:128)] = W_pair^T @ x^T,
    so there are no PE transposes at all.
  - Per-token variance: ACT Square of the pair psum -> sq in SBUF, then
    PE ones-matmuls reduce over the partition (d) axis:
      ssq (q rows) in row layout [1, N] for the q-side broadcast,
      ssk (k rows) in column layout [128, 16] for the exp scale.
  - rstd = exp(-0.5*ln(var)) on ACT: ln/exp/square/copy all live in the
    natural_log_exp_and_others table set -> zero table switches.
  - q_hat = q_centered * rstd_q via one gpsimd row-broadcast + one DVE
    multiply; k needs NO elementwise work: 0.125*rstd_k is folded into
    the exp's per-partition scale (softmax rows are k-tokens).
  - exp(0.125*rstd_k*s - 4) directly; qk-normed scores are O(1) so no
    max-subtract is needed.
  - AV with a ones-column in v yields softmax row sums; out^T rows are
    normalized with a reciprocal+broadcast chain overlapped with the
    next head; projection y = oT^T @ W_proj rows with K=192.
"""

import contextlib
import sys

import numpy as np

sys.path.insert(0, "/opt/trn_rl_repo")

import ml_dtypes

import concourse.bass as bass
import concourse.tile as tile
from concourse import bacc, bass_utils, mybir
from concourse.masks import make_identity

BF16 = mybir.dt.bfloat16
F32 = mybir.dt.float32
I16 = mybir.dt.int16

B, N, C = 2, 2048, 768
H, D = 12, 64
HL = 3          # heads per core
P = 128
NT = N // P     # 16 token tiles
KC = C // P     # 6 contraction tiles over C
QC = 512
EPS = 1e-5
EXP_SHIFT = -4.0
SCALE = D ** -0.5  # 0.125


def _build(nc, apply_gb):
    """Emit the per-core program (SPMD: all 8 cores run this graph)."""
    xt_d = nc.dram_tensor("xt", [C, N], BF16, kind="ExternalInput")
    wqk_d = nc.dram_tensor("wqk", [C, HL * P], BF16, kind="ExternalInput")
    wv_d = nc.dram_tensor("wv", [C, HL * D], BF16, kind="ExternalInput")
    wp_d = nc.dram_tensor("wp", [HL * D, C], BF16, kind="ExternalInput")
    gb_d = nc.dram_tensor("gb", [D, 12], F32, kind="ExternalInput")
    out_d = nc.dram_tensor("out", [N, C], BF16, kind="ExternalOutput")

    AF = mybir.ActivationFunctionType

    with tile.TileContext(nc) as tc:
        ctx = contextlib.ExitStack()
        with ctx:
            singles = ctx.enter_context(tc.tile_pool(name="singles", bufs=1))
            persist = ctx.enter_context(tc.tile_pool(name="persist", bufs=1))

            # ---- weights / x^T loads (xt per kc so pair0 can start early;
            # interleaved so the first slices land first) ----
            wqk_sb = persist.tile([P, KC, HL * P], BF16)
            xt_sb = persist.tile([P, KC, N], BF16)
            for kc in range(KC):
                ksl = slice(kc * P, (kc + 1) * P)
                nc.sync.dma_start(out=wqk_sb[:, kc, :], in_=wqk_d.ap()[ksl, :])
                nc.sync.dma_start(out=xt_sb[:, kc, :], in_=xt_d.ap()[ksl, :])
            wv_sb = persist.tile([P, KC, HL * D], BF16)
            for kc in range(KC):
                ksl = slice(kc * P, (kc + 1) * P)
                nc.gpsimd.dma_start(out=wv_sb[:, kc, :], in_=wv_d.ap()[ksl, :])
            wpA = persist.tile([P, C], BF16)
            nc.gpsimd.dma_start(out=wpA, in_=wp_d.ap()[0:P, :])
            wpB = persist.tile([64, C], BF16)
            nc.gpsimd.dma_start(out=wpB, in_=wp_d.ap()[P : P + 64, :])
            gb_sb = singles.tile([64, 12], F32)
            nc.gpsimd.dma_start(out=gb_sb, in_=gb_d.ap())

            # ---- constants ----
            shift_t = singles.tile([P, 1], F32)
            nc.vector.memset(shift_t, EXP_SHIFT)
            zero_t = singles.tile([P, 1], F32)
            nc.vector.memset(zero_t, 0.0)
            epsq_t = singles.tile([P, 1], F32)
            nc.vector.memset(epsq_t, EPS)
            epsk_t = singles.tile([P, 1], F32)
            nc.vector.memset(epsk_t, 64.0 * EPS)
            ones64 = singles.tile([64, 1], BF16)
            nc.vector.memset(ones64, 1.0)
            ident16 = singles.tile([NT, NT], F32)
            make_identity(nc, ident16)
            onesk = singles.tile([P, 1], BF16)
            nc.vector.memset(onesk[0:64, :], 0.0)
            nc.vector.memset(onesk[64:P, :], 1.0)

            # ---- persistent activations ----
            ksb = [
                persist.tile([64, N], BF16, tag=f"ksb{h}", name=f"ksb{h}")
                for h in range(HL)
            ]
            qhat = [
                persist.tile([64, N], BF16, tag=f"qhat{h}", name=f"qhat{h}")
                for h in range(HL)
            ]
            rk_cols = persist.tile([P, HL, NT], F32, tag="rk_cols")
            rkA_cols = persist.tile([P, HL, NT], F32, tag="rkA_cols")
            v_all = persist.tile([P, NT, HL, 66], BF16, tag="v_all")
            nc.gpsimd.memset(v_all[:, :, :, 64:65], 1.0)
            oTA = persist.tile([P, N], BF16, tag="oTA")
            oTB = persist.tile([64, N], BF16, tag="oTB")

            # ============ phase B: qk pairs + stats + v ============
            with tc.tile_pool(name="pairp", bufs=1, space="PSUM") as pairp, \
                 tc.tile_pool(name="ssqp", bufs=1, space="PSUM") as ssqp, \
                 tc.tile_pool(name="vp", bufs=1, space="PSUM") as vp, \
                 tc.tile_pool(name="kstp", bufs=1) as kstp, \
                 tc.tile_pool(name="qrawp", bufs=2) as qrawp, \
                 tc.tile_pool(name="sqp", bufs=2) as sqp, \
                 tc.tile_pool(name="rqp", bufs=2) as rqp, \
                 tc.tile_pool(name="rbp", bufs=1) as rbp:

                vb_count = [0]

                def _emit_v_batch():
                    # two token tiles per batch; v_ps fits one PSUM bank
                    vb = vb_count[0]
                    if vb >= NT // 2:
                        return
                    vb_count[0] += 1
                    v_ps = vp.tile([P, 2, HL * D], F32, tag="v_ps")
                    for m in range(2):
                        mt = 2 * vb + m
                        msl = slice(mt * P, (mt + 1) * P)
                        for kc in range(KC):
                            nc.tensor.matmul(
                                v_ps[:, m, :], xt_sb[:, kc, msl],
                                wv_sb[:, kc, :],
                                start=(kc == 0), stop=(kc == KC - 1),
                            )
                    nc.vector.tensor_copy(
                        v_all[:, 2 * vb : 2 * vb + 2, :, 0:64],
                        v_ps[:].rearrange("p b (h d) -> p b h d", h=HL),
                    )

                rk16s = []
                for h in range(HL):
                    hsl = slice(h * P, (h + 1) * P)
                    pair = pairp.tile([P, N], F32, tag="pair")
                    if h == 0:
                        # warm fillers gated on each xt slice arrival keep
                        # the PE active through the load phase (HAM ramps
                        # the clock only under sustained activity)
                        nwarm = [5, 5, 4, 3, 2, 1]
                        for kc in range(KC):
                            for j in range(nwarm[kc]):
                                warm = ssqp.tile([1, QC], F32, tag="ssq")
                                nc.tensor.matmul(
                                    warm,
                                    xt_sb[:, kc, 0:1], xt_sb[:, kc, 0:QC],
                                    start=True, stop=True,
                                )
                            for q4 in range(4):
                                nc.tensor.matmul(
                                    pair[:, q4 * QC : (q4 + 1) * QC],
                                    wqk_sb[:, kc, hsl],
                                    xt_sb[:, kc, q4 * QC : (q4 + 1) * QC],
                                    start=(kc == 0), stop=(kc == KC - 1),
                                )
                    else:
                        for kc in range(KC):
                            for q4 in range(4):
                                nc.tensor.matmul(
                                    pair[:, q4 * QC : (q4 + 1) * QC],
                                    wqk_sb[:, kc, hsl],
                                    xt_sb[:, kc, q4 * QC : (q4 + 1) * QC],
                                    start=(kc == 0), stop=(kc == KC - 1),
                                )

                    # squares for the variance reductions (rb chain head)
                    sq = sqp.tile([P, N], BF16, tag="sq")
                    nc.scalar.activation(sq, pair, func=AF.Square, bias=zero_t)
                    # k: psum -> staged copy -> DMA partition-shift to base 0
                    kst = kstp.tile([P, N], BF16, tag="kst")
                    nc.vector.tensor_copy(kst[64:P, :], pair[64:P, :])
                    nc.sync.dma_start(out=ksb[h], in_=kst[64:P, :])
                    # q raw out of psum (frees the pair psum for pair h+1)
                    qraw = qrawp.tile([64, N], BF16, tag="qraw")
                    nc.vector.tensor_copy(qraw, pair[0:64, :])

                    # v batches fill the PE while ACT computes the squares
                    _emit_v_batch()
                    _emit_v_batch()

                    # ssq/ssk: row-layout sums of squares over the 64 q (k)
                    # partitions; rstd = (ss*scale + eps)^-1/2 on ACT.
                    # abs_reciprocal_sqrt shares its table set with
                    # square/copy, so phase B needs no ACT table switches.
                    rq = rqp.tile([1, N], F32, tag="rq")
                    rk_row = rqp.tile([1, N], F32, tag="rk_row")
                    for qt in range(4):
                        fsl = slice(qt * QC, (qt + 1) * QC)
                        ssq = ssqp.tile([1, QC], F32, tag="ssq")
                        ssk = ssqp.tile([1, QC], F32, tag="ssk")
                        nc.tensor.matmul(
                            ssq, ones64, sq[0:64, fsl],
                            start=True, stop=True,
                        )
                        nc.tensor.matmul(
                            ssk, onesk[64:P, :], sq[64:P, fsl],
                            start=True, stop=True,
                        )
                        nc.scalar.activation(
                            rq[:, fsl], ssq, func=AF.Abs_reciprocal_sqrt,
                            bias=epsq_t[0:1, :], scale=1.0 / 64,
                        )
                        # 0.125*rstd_k = (ssk + 64*eps)^-1/2
                        nc.scalar.activation(
                            rk_row[:, fsl], ssk, func=AF.Abs_reciprocal_sqrt,
                            bias=epsk_t[0:1, :], scale=1.0,
                        )
                    # reshape the rstd_k row to [16, 128]; a single PE
                    # transpose at the end of phase B makes it [128, 16]
                    rk16 = persist.tile(
                        [NT, P], F32, tag=f"rk16_{h}", name=f"rk16_{h}"
                    )
                    nc.gpsimd.dma_start(out=rk16, in_=rk_row)
                    rk16s.append(rk16)

                    # broadcast rstd_q across 64 partitions, apply to q
                    rb = rbp.tile([64, N], F32, tag="rb")
                    for half in range(2):
                        fsl = slice(half * N // 2, (half + 1) * N // 2)
                        nc.gpsimd.partition_broadcast(
                            rb[:, fsl], rq[:, fsl], channels=64
                        )
                    nc.vector.tensor_tensor(
                        out=qhat[h], in0=qraw, in1=rb,
                        op=mybir.AluOpType.mult,
                    )
                    if apply_gb:
                        # general gamma/beta: per-partition affines; k also
                        # needs rstd_k applied elementwise (exp scale is
                        # the plain 0.125 const in this mode)
                        nc.vector.tensor_scalar(
                            qhat[h], qhat[h],
                            gb_sb[:, h : h + 1], gb_sb[:, 6 + h : 7 + h],
                            op0=mybir.AluOpType.mult,
                            op1=mybir.AluOpType.add,
                        )
                        rk2 = rqp.tile([1, N], F32, tag="rq")
                        for qt in range(4):
                            fsl = slice(qt * QC, (qt + 1) * QC)
                            ssk2 = ssqp.tile([1, QC], F32, tag="ssk")
                            nc.tensor.matmul(
                                ssk2, onesk[64:P, :], sq[64:P, fsl],
                                start=True, stop=True,
                            )
                            nc.scalar.activation(
                                rk2[:, fsl], ssk2,
                                func=AF.Abs_reciprocal_sqrt,
                                bias=epsq_t[0:1, :], scale=1.0 / 64,
                            )
                        rbk = rbp.tile([64, N], F32, tag="rb")
                        for half in range(2):
                            fsl = slice(half * N // 2, (half + 1) * N // 2)
                            nc.gpsimd.partition_broadcast(
                                rbk[:, fsl], rk2[:, fsl], channels=64
                            )
                        nc.vector.tensor_tensor(
                            out=ksb[h], in0=ksb[h], in1=rbk,
                            op=mybir.AluOpType.mult,
                        )
                        nc.vector.tensor_scalar(
                            ksb[h], ksb[h],
                            gb_sb[:, 3 + h : 4 + h], gb_sb[:, 9 + h : 10 + h],
                            op0=mybir.AluOpType.mult,
                            op1=mybir.AluOpType.add,
                        )

                    # v batches fill the PE while the stat chains drain
                    _emit_v_batch()
                    _emit_v_batch()
                while vb_count[0] < NT // 2:
                    _emit_v_batch()
                # one PE transpose per head turns the [16, 128] rstd_k
                # reshape into the [128, 16] exp-scale column form
                for h in range(HL):
                    rkT = vp.tile([P, NT], F32, tag="rkT")
                    nc.tensor.transpose(rkT, rk16s[h], ident16)
                    nc.scalar.copy(rk_cols[:, h, :], rkT)
                    # bit-trick exp scale for the DVE half: A = 128*log2(e)*s
                    nc.vector.tensor_scalar_mul(
                        rkA_cols[:, h, :], rkT, 184.6650559
                    )

            # ================= attention =================
            with tc.tile_pool(name="scps", bufs=2, space="PSUM") as psc, \
                 tc.tile_pool(name="avps", bufs=1, space="PSUM") as pav, \
                 tc.tile_pool(name="expsb", bufs=3) as pexp, \
                 tc.tile_pool(name="avfsb", bufs=1) as pavf, \
                 tc.tile_pool(name="rrsb", bufs=2) as prr, \
                 tc.tile_pool(name="rbnsb", bufs=1) as prbn, \
                 tc.tile_pool(name="o1sb", bufs=1) as po1:

                def sc_mms(kT, qT, kt, half):
                    sct = psc.tile([P, 2 * QC], F32, tag="sc")
                    for q2 in range(2):
                        qsl = slice((2 * half + q2) * QC, (2 * half + q2 + 1) * QC)
                        nc.tensor.matmul(
                            sct[:, q2 * QC : (q2 + 1) * QC],
                            kT[:, kt * P : (kt + 1) * P], qT[:, qsl],
                            start=True, stop=True,
                        )
                    return sct

                s0 = s1 = None
                for h in range(HL):
                    kT, qT = ksb[h], qhat[h]
                    escale = rk_cols[:, h, :]
                    escaleA = rkA_cols[:, h, :]
                    av_ps = pav.tile([65, N], F32, tag="av")
                    if s0 is None:
                        s0 = sc_mms(kT, qT, 0, 0)
                        s1 = sc_mms(kT, qT, 0, 1)
                    for kt in range(NT):
                        eT = pexp.tile([P, N], BF16, tag="expT")
                        nc.scalar.activation(
                            eT[:, 0:1024], s0, func=AF.Exp,
                            bias=shift_t,
                            scale=(escale[:, kt : kt + 1] if not apply_gb
                                   else SCALE),
                        )
                        # DVE computes the other half with a Schraudolph
                        # bit-trick exp: the int16 convert of A*s + B IS the
                        # bf16 pattern of exp(scale*s + shift) (~1.8% rms,
                        # verified end-to-end well under the error budget)
                        nc.vector.tensor_scalar(
                            eT[:, 1024:2048].bitcast(I16), s1,
                            (escaleA[:, kt : kt + 1] if not apply_gb
                             else 23.0831),
                            15510.336,
                            op0=mybir.AluOpType.mult,
                            op1=mybir.AluOpType.add,
                        )
                        # keep-warm fillers into the dead sc tile
                        nj = 1 if kt % 2 == 0 else 0
                        for j in range(nj):
                            nc.tensor.matmul(
                                s0[:, 0:QC],
                                xt_sb[:, 0, 0:P], xt_sb[:, 0, 0:QC],
                                start=True, stop=True,
                            )
                        if kt < NT - 1:
                            s0n = sc_mms(kT, qT, kt + 1, 0)
                        for qc in (0, 1):
                            nc.tensor.matmul(
                                av_ps[:, qc * QC : (qc + 1) * QC],
                                v_all[:, kt, h, 0:65],
                                eT[:, qc * QC : (qc + 1) * QC],
                                start=(kt == 0), stop=(kt == NT - 1),
                            )
                        if kt < NT - 1:
                            s1n = sc_mms(kT, qT, kt + 1, 1)
                        for qc in (2, 3):
                            nc.tensor.matmul(
                                av_ps[:, qc * QC : (qc + 1) * QC],
                                v_all[:, kt, h, 0:65],
                                eT[:, qc * QC : (qc + 1) * QC],
                                start=(kt == 0), stop=(kt == NT - 1),
                            )
                        if kt < NT - 1:
                            s0, s1 = s0n, s1n
                    # next head's score prologue first so ACT never starves
                    if h + 1 < HL:
                        for j in range(2):
                            nc.tensor.matmul(
                                s1[:, 0:QC],
                                xt_sb[:, 0, 0:P], xt_sb[:, 0, 0:QC],
                                start=True, stop=True,
                            )
                        s0 = sc_mms(ksb[h + 1], qhat[h + 1], 0, 0)
                        s1 = sc_mms(ksb[h + 1], qhat[h + 1], 0, 1)

                    if h < 2:
                        # drain AV psum; normalize overlaps the next head
                        avf = pavf.tile([65, N], F32, tag="avf")
                        nc.vector.tensor_copy(avf, av_ps)
                        # 1/rowsum: [1,2048] -> [4,512] (DMA), recip, back
                        s4 = prr.tile([4, QC], F32, tag="s4")
                        nc.gpsimd.dma_start(out=s4, in_=avf[64:65, :])
                        r4 = prr.tile([4, QC], F32, tag="r4")
                        nc.vector.reciprocal_approx_fast(out=r4, in_=s4)
                        rr = prr.tile([1, N], F32, tag="rr")
                        nc.gpsimd.dma_start(out=rr, in_=r4)
                        rbn = prbn.tile([64, N], F32, tag="rbn")
                        for half in range(2):
                            fsl = slice(half * N // 2, (half + 1) * N // 2)
                            nc.gpsimd.partition_broadcast(
                                rbn[:, fsl], rr[:, fsl], channels=64
                            )
                        if h == 0:
                            nc.vector.tensor_tensor(
                                out=oTA[0:64, :], in0=avf[0:64, :], in1=rbn,
                                op=mybir.AluOpType.mult,
                            )
                        else:
                            # DVE cannot shift partitions; write base-0
                            # tmp then DMA into oTA rows 64-127
                            tmp = po1.tile([64, N], BF16, tag="o1tmp")
                            nc.vector.tensor_tensor(
                                out=tmp, in0=avf[0:64, :], in1=rbn,
                                op=mybir.AluOpType.mult,
                            )
                            nc.sync.dma_start(out=oTA[64:P, :], in_=tmp)
                    else:
                        # h2 gates the projection: process in q-halves so
                        # proj mts 0-7 start while half 1 is in flight
                        avf = pavf.tile([65, N], F32, tag="avf")
                        for half in range(2):
                            fsl = slice(half * 1024, (half + 1) * 1024)
                            nc.vector.tensor_copy(avf[:, fsl], av_ps[:, fsl])
                            s2 = prr.tile([2, QC], F32, tag="s4")
                            nc.gpsimd.dma_start(out=s2, in_=avf[64:65, fsl])
                            r2 = prr.tile([2, QC], F32, tag="r4")
                            nc.vector.reciprocal_approx_fast(out=r2, in_=s2)
                            rr2 = prr.tile([1, N], F32, tag="rr")
                            nc.gpsimd.dma_start(out=rr2[:, fsl], in_=r2)
                            rbn2 = prbn.tile([64, N], F32, tag="rbn")
                            nc.gpsimd.partition_broadcast(
                                rbn2[:, fsl], rr2[:, fsl], channels=64
                            )
                            nc.vector.tensor_tensor(
                                out=oTB[:, fsl], in0=avf[0:64, fsl],
                                in1=rbn2[:, fsl],
                                op=mybir.AluOpType.mult,
                            )

            # ================= projection =================
            with tc.tile_pool(name="pjps", bufs=3, space="PSUM") as ppj, \
                 tc.tile_pool(name="pjw", bufs=2, space="PSUM") as ppw, \
                 tc.tile_pool(name="ysb", bufs=3) as py:
                for mt in range(NT):
                    msl = slice(mt * P, (mt + 1) * P)
                    y_ps = ppj.tile([P, C], F32, tag="y")
                    # warm filler keeps the HAM activity monitor fed so the
                    # PE stays at full clock through the projection
                    warmp = ppw.tile([P, P], F32, tag="warmp")
                    nc.tensor.matmul(
                        warmp, xt_sb[:, 0, 0:P], xt_sb[:, 0, 0:P],
                        start=True, stop=True,
                    )
                    for n0, n1 in [(0, 512), (512, 768)]:
                        nc.tensor.matmul(
                            y_ps[:, n0:n1], oTA[:, msl], wpA[:, n0:n1],
                            start=True, stop=False,
                        )
                        nc.tensor.matmul(
                            y_ps[:, n0:n1], oTB[:, msl], wpB[:, n0:n1],
                            start=False, stop=True,
                        )
                    y_out = py.tile([P, C], BF16, tag="y_out")
                    # split the drain across both copy engines
                    nc.vector.tensor_copy(y_out[:, 0:384], y_ps[:, 0:384])
                    nc.scalar.copy(y_out[:, 384:768], y_ps[:, 384:768])
                    nc.sync.dma_start(out=out_d.ap()[msl, :], in_=y_out)

    nc.compile()
    return nc


_CACHED = {}


def _get_nc(apply_gb):
    key = ("nc", apply_gb)
    if key not in _CACHED:
        nc = bacc.Bacc("TRN2", target_bir_lowering=False, debug=False)
        _CACHED[key] = _build(nc, apply_gb)
    return _CACHED[key]


def _make_in_maps(inputs):
    x = np.asarray(inputs["x"], np.float32)
    wqkv = np.asarray(inputs["W_qkv"], np.float32)
    wproj = np.asarray(inputs["W_proj"], np.float32)
    qg = np.asarray(inputs["q_gamma"], np.float32)
    qb = np.asarray(inputs["q_beta"], np.float32)
    kg = np.asarray(inputs["k_gamma"], np.float32)
    kb = np.asarray(inputs["k_beta"], np.float32)

    bf = ml_dtypes.bfloat16
    w3 = wqkv.reshape(C, 3, H, D)
    cmat = np.eye(D, dtype=np.float32) - np.full((D, D), 1.0 / D, np.float32)
    in_maps = []
    for c in range(8):
        b = c // 4
        h0 = (c % 4) * HL
        cols = []
        for hh in range(HL):
            cols.append(w3[:, 0, h0 + hh, :] @ cmat)  # centered Wq
            cols.append(w3[:, 1, h0 + hh, :] @ cmat)  # centered Wk
        wqk = np.concatenate(cols, axis=1)  # [C, 384]
        wv = np.ascontiguousarray(
            w3[:, 2, h0 : h0 + HL, :].reshape(C, HL * D)
        )
        gbm = np.zeros((D, 12), np.float32)
        gbm[:, 0:3] = qg[:, None]
        gbm[:, 3:6] = kg[:, None]
        gbm[:, 6:9] = qb[:, None]
        gbm[:, 9:12] = kb[:, None]
        in_maps.append(
            {
                "xt": np.ascontiguousarray(x[b].T).astype(bf),
                "wqk": np.ascontiguousarray(wqk).astype(bf),
                "wv": wv.astype(bf),
                "wp": np.ascontiguousarray(
                    wproj[h0 * D : (h0 + HL) * D, :]
                ).astype(bf),
                "gb": gbm,
            }
        )
    return in_maps


def _gather(inputs, results):
    bproj = np.asarray(inputs["b_proj"], np.float32)
    y = np.zeros((B, N, C), np.float32)
    for c in range(8):
        y[c // 4] += np.asarray(results[c]["out"], dtype=np.float32)
    y += bproj
    return y


def _install_profile_hook():
    """The agent image's antenv lacks axon_hooks; synthesize it so
    run_bass_kernel_spmd(trace=True) can NTFF-profile via ctypes."""
    import types

    if "antenv.axon_hooks" in sys.modules:
        return
    try:
        from trn_agent_boot.trn_boot import _ntff_profile_via_ctypes

        hook = _ntff_profile_via_ctypes("/opt/axon/libaxon_pjrt.so")
    except Exception:
        hook = None
    mod = types.ModuleType("antenv.axon_hooks")
    mod.get_axon_ntff_profile_hook = lambda: hook
    mod.set_axon_ntff_profile_hook = lambda h: None
    sys.modules["antenv.axon_hooks"] = mod
    # no S3 in this container: keep artifacts local
    bass_utils.upload_artifacts = lambda tmpdir: tmpdir


def _kernel_impl(inputs, trace=False, tmpdir=None):
    apply_gb = not (
        np.all(np.asarray(inputs["q_gamma"]) == 1.0)
        and np.all(np.asarray(inputs["k_gamma"]) == 1.0)
        and np.all(np.asarray(inputs["q_beta"]) == 0.0)
        and np.all(np.asarray(inputs["k_beta"]) == 0.0)
    )
    nc = _get_nc(apply_gb)
    in_maps = _make_in_maps(inputs)
    if trace:
        _install_profile_hook()
    res = bass_utils.run_bass_kernel_spmd(
        nc, in_maps, core_ids=list(range(8)), trace=trace, tmpdir=tmpdir
    )
    out = _gather(inputs, res.results)
    return out, res


def kernel(**inputs):
    out, _ = _kernel_impl(inputs)
    return out


def kernel_with_profile(**inputs):
    out, res = _kernel_impl(inputs, trace=True)
    return out, res


# revision 18
# speedup vs baseline: 1.2895x; 1.0175x over previous
"""Distributed Trainium2 kernel for qk-norm attention (restructured).

Reference computation (B=2, N=2048, C=768, H=12, D=64):
    qkv = x @ W_qkv; q,k,v split per head
    q = LN(q)*scale, k = LN(k)   (LN over head_dim, with gamma/beta)
    out = softmax(q k^T) v ; y = concat_heads(out) @ W_proj + b_proj

Sharding: 24 (batch, head) units -> 8 cores: core c handles batch c//4
and heads 3*(c%4) .. 3*(c%4)+2.  Each core computes a partial
projection y_partial = out_heads @ W_proj[rows]; the host sums the 4
partials per batch and adds b_proj.

Key structural ideas (vs a natural-layout qkv + LN + transpose design):
  - LN centering is linear, so it is folded into the weights host-side:
    W_hat = W_qk @ (I - J/64).  The device never computes means.
  - q/k are computed DIRECTLY in transposed [d, token] layout:
    pair_h = [q_h (partitions 0:64) | k_h (64:128)] = W_pair^T @ x^T,
    so there are no PE transposes at all.
  - Per-token variance: ACT Square of the pair psum -> sq in SBUF, then
    PE ones-matmuls reduce over the partition (d) axis:
      ssq (q rows) in row layout [1, N] for the q-side broadcast,
      ssk (k rows) in column layout [128, 16] for the exp scale.
  - rstd = exp(-0.5*ln(var)) on ACT: ln/exp/square/copy all live in the
    natural_log_exp_and_others table set -> zero table switches.
  - q_hat = q_centered * rstd_q via one gpsimd row-broadcast + one DVE
    multiply; k needs NO elementwise work: 0.125*rstd_k is folded into
    the exp's per-partition scale (softmax rows are k-tokens).
  - exp(0.125*rstd_k*s - 4) directly; qk-normed scores are O(1) so no
    max-subtract is needed.
  - AV with a ones-column in v yields softmax row sums; out^T rows are
    normalized with a reciprocal+broadcast chain overlapped with the
    next head; projection y = oT^T @ W_proj rows with K=192.
"""

import contextlib
import sys

import numpy as np

sys.path.insert(0, "/opt/trn_rl_repo")

import ml_dtypes

import concourse.bass as bass
import concourse.tile as tile
from concourse import bacc, bass_utils, mybir
from concourse.masks import make_identity

BF16 = mybir.dt.bfloat16
F32 = mybir.dt.float32
I16 = mybir.dt.int16

B, N, C = 2, 2048, 768
H, D = 12, 64
HL = 3          # heads per core
P = 128
NT = N // P     # 16 token tiles
KC = C // P     # 6 contraction tiles over C
QC = 512
EPS = 1e-5
EXP_SHIFT = -4.0
SCALE = D ** -0.5  # 0.125


def _build(nc, apply_gb):
    """Emit the per-core program (SPMD: all 8 cores run this graph)."""
    xt_d = nc.dram_tensor("xt", [C, N], BF16, kind="ExternalInput")
    wqk_d = nc.dram_tensor("wqk", [C, HL * P], BF16, kind="ExternalInput")
    wv_d = nc.dram_tensor("wv", [C, HL * D], BF16, kind="ExternalInput")
    wp_d = nc.dram_tensor("wp", [HL * D, C], BF16, kind="ExternalInput")
    gb_d = nc.dram_tensor("gb", [D, 12], F32, kind="ExternalInput")
    out_d = nc.dram_tensor("out", [N, C], BF16, kind="ExternalOutput")

    AF = mybir.ActivationFunctionType

    with tile.TileContext(nc) as tc:
        ctx = contextlib.ExitStack()
        with ctx:
            singles = ctx.enter_context(tc.tile_pool(name="singles", bufs=1))
            persist = ctx.enter_context(tc.tile_pool(name="persist", bufs=1))

            # ---- weights / x^T loads (xt per kc so pair0 can start early;
            # interleaved so the first slices land first) ----
            wqk_sb = persist.tile([P, KC, HL * P], BF16)
            xt_sb = persist.tile([P, KC, N], BF16)
            for kc in range(KC):
                ksl = slice(kc * P, (kc + 1) * P)
                nc.sync.dma_start(out=wqk_sb[:, kc, :], in_=wqk_d.ap()[ksl, :])
                nc.sync.dma_start(out=xt_sb[:, kc, :], in_=xt_d.ap()[ksl, :])
            wv_sb = persist.tile([P, KC, HL * D], BF16)
            for kc in range(KC):
                ksl = slice(kc * P, (kc + 1) * P)
                nc.gpsimd.dma_start(out=wv_sb[:, kc, :], in_=wv_d.ap()[ksl, :])
            wpA = persist.tile([P, C], BF16)
            nc.gpsimd.dma_start(out=wpA, in_=wp_d.ap()[0:P, :])
            wpB = persist.tile([64, C], BF16)
            nc.gpsimd.dma_start(out=wpB, in_=wp_d.ap()[P : P + 64, :])
            gb_sb = singles.tile([64, 12], F32)
            nc.gpsimd.dma_start(out=gb_sb, in_=gb_d.ap())

            # ---- constants ----
            shift_t = singles.tile([P, 1], F32)
            nc.vector.memset(shift_t, EXP_SHIFT)
            zero_t = singles.tile([P, 1], F32)
            nc.vector.memset(zero_t, 0.0)
            epsq_t = singles.tile([P, 1], F32)
            nc.vector.memset(epsq_t, EPS)
            epsk_t = singles.tile([P, 1], F32)
            nc.vector.memset(epsk_t, 64.0 * EPS)
            ones64 = singles.tile([64, 1], BF16)
            nc.vector.memset(ones64, 1.0)
            ident16 = singles.tile([NT, NT], F32)
            make_identity(nc, ident16)
            onesk = singles.tile([P, 1], BF16)
            nc.vector.memset(onesk[0:64, :], 0.0)
            nc.vector.memset(onesk[64:P, :], 1.0)

            # ---- persistent activations ----
            ksb = [
                persist.tile([64, N], BF16, tag=f"ksb{h}", name=f"ksb{h}")
                for h in range(HL)
            ]
            qhat = [
                persist.tile([64, N], BF16, tag=f"qhat{h}", name=f"qhat{h}")
                for h in range(HL)
            ]
            rk_cols = persist.tile([P, HL, NT], F32, tag="rk_cols")
            rkA_cols = persist.tile([P, HL, NT], F32, tag="rkA_cols")
            v_all = persist.tile([P, NT, HL, 66], BF16, tag="v_all")
            nc.gpsimd.memset(v_all[:, :, :, 64:65], 1.0)
            oTA = persist.tile([P, N], BF16, tag="oTA")
            oTB = persist.tile([64, N], BF16, tag="oTB")

            # ============ phase B: qk pairs + stats + v ============
            with tc.tile_pool(name="pairp", bufs=1, space="PSUM") as pairp, \
                 tc.tile_pool(name="ssqp", bufs=1, space="PSUM") as ssqp, \
                 tc.tile_pool(name="vp", bufs=1, space="PSUM") as vp, \
                 tc.tile_pool(name="kstp", bufs=1) as kstp, \
                 tc.tile_pool(name="qrawp", bufs=2) as qrawp, \
                 tc.tile_pool(name="sqp", bufs=2) as sqp, \
                 tc.tile_pool(name="rqp", bufs=2) as rqp, \
                 tc.tile_pool(name="rbp", bufs=1) as rbp:

                vb_count = [0]

                def _emit_v_batch():
                    # two token tiles per batch; v_ps fits one PSUM bank
                    vb = vb_count[0]
                    if vb >= NT // 2:
                        return
                    vb_count[0] += 1
                    v_ps = vp.tile([P, 2, HL * D], F32, tag="v_ps")
                    for m in range(2):
                        mt = 2 * vb + m
                        msl = slice(mt * P, (mt + 1) * P)
                        for kc in range(KC):
                            nc.tensor.matmul(
                                v_ps[:, m, :], xt_sb[:, kc, msl],
                                wv_sb[:, kc, :],
                                start=(kc == 0), stop=(kc == KC - 1),
                            )
                    nc.vector.tensor_copy(
                        v_all[:, 2 * vb : 2 * vb + 2, :, 0:64],
                        v_ps[:].rearrange("p b (h d) -> p b h d", h=HL),
                    )

                rk16s = []
                for h in range(HL):
                    hsl = slice(h * P, (h + 1) * P)
                    pair = pairp.tile([P, N], F32, tag="pair")
                    if h == 0:
                        # warm fillers gated on each xt slice arrival keep
                        # the PE active through the load phase (HAM ramps
                        # the clock only under sustained activity)
                        nwarm = [5, 5, 4, 3, 2, 1]
                        for kc in range(KC):
                            for j in range(nwarm[kc]):
                                warm = ssqp.tile([1, QC], F32, tag="ssq")
                                nc.tensor.matmul(
                                    warm,
                                    xt_sb[:, kc, 0:1], xt_sb[:, kc, 0:QC],
                                    start=True, stop=True,
                                )
                            for q4 in range(4):
                                nc.tensor.matmul(
                                    pair[:, q4 * QC : (q4 + 1) * QC],
                                    wqk_sb[:, kc, hsl],
                                    xt_sb[:, kc, q4 * QC : (q4 + 1) * QC],
                                    start=(kc == 0), stop=(kc == KC - 1),
                                )
                    else:
                        for kc in range(KC):
                            for q4 in range(4):
                                nc.tensor.matmul(
                                    pair[:, q4 * QC : (q4 + 1) * QC],
                                    wqk_sb[:, kc, hsl],
                                    xt_sb[:, kc, q4 * QC : (q4 + 1) * QC],
                                    start=(kc == 0), stop=(kc == KC - 1),
                                )

                    # squares for the variance reductions (rb chain head)
                    sq = sqp.tile([P, N], BF16, tag="sq")
                    nc.scalar.activation(sq, pair, func=AF.Square, bias=zero_t)
                    # k: psum -> staged copy -> DMA partition-shift to base 0
                    kst = kstp.tile([P, N], BF16, tag="kst")
                    nc.vector.tensor_copy(kst[64:P, :], pair[64:P, :])
                    nc.sync.dma_start(out=ksb[h], in_=kst[64:P, :])
                    # q raw out of psum (frees the pair psum for pair h+1)
                    qraw = qrawp.tile([64, N], BF16, tag="qraw")
                    nc.vector.tensor_copy(qraw, pair[0:64, :])

                    # v batches fill the PE while ACT computes the squares
                    _emit_v_batch()
                    _emit_v_batch()

                    # ssq/ssk: row-layout sums of squares over the 64 q (k)
                    # partitions; rstd = (ss*scale + eps)^-1/2 on ACT.
                    # abs_reciprocal_sqrt shares its table set with
                    # square/copy, so phase B needs no ACT table switches.
                    rq = rqp.tile([1, N], F32, tag="rq")
                    rk_row = rqp.tile([1, N], F32, tag="rk_row")
                    for qt in range(4):
                        fsl = slice(qt * QC, (qt + 1) * QC)
                        ssq = ssqp.tile([1, QC], F32, tag="ssq")
                        ssk = ssqp.tile([1, QC], F32, tag="ssk")
                        nc.tensor.matmul(
                            ssq, ones64, sq[0:64, fsl],
                            start=True, stop=True,
                        )
                        nc.tensor.matmul(
                            ssk, onesk[64:P, :], sq[64:P, fsl],
                            start=True, stop=True,
                        )
                        nc.scalar.activation(
                            rq[:, fsl], ssq, func=AF.Abs_reciprocal_sqrt,
                            bias=epsq_t[0:1, :], scale=1.0 / 64,
                        )
                        # 0.125*rstd_k = (ssk + 64*eps)^-1/2
                        nc.scalar.activation(
                            rk_row[:, fsl], ssk, func=AF.Abs_reciprocal_sqrt,
                            bias=epsk_t[0:1, :], scale=1.0,
                        )
                    # reshape the rstd_k row to [16, 128]; a single PE
                    # transpose at the end of phase B makes it [128, 16]
                    rk16 = persist.tile(
                        [NT, P], F32, tag=f"rk16_{h}", name=f"rk16_{h}"
                    )
                    nc.gpsimd.dma_start(out=rk16, in_=rk_row)
                    rk16s.append(rk16)

                    # broadcast rstd_q across 64 partitions, apply to q
                    rb = rbp.tile([64, N], F32, tag="rb")
                    for half in range(2):
                        fsl = slice(half * N // 2, (half + 1) * N // 2)
                        nc.gpsimd.partition_broadcast(
                            rb[:, fsl], rq[:, fsl], channels=64
                        )
                    nc.vector.tensor_tensor(
                        out=qhat[h], in0=qraw, in1=rb,
                        op=mybir.AluOpType.mult,
                    )
                    if apply_gb:
                        # general gamma/beta: per-partition affines; k also
                        # needs rstd_k applied elementwise (exp scale is
                        # the plain 0.125 const in this mode)
                        nc.vector.tensor_scalar(
                            qhat[h], qhat[h],
                            gb_sb[:, h : h + 1], gb_sb[:, 6 + h : 7 + h],
                            op0=mybir.AluOpType.mult,
                            op1=mybir.AluOpType.add,
                        )
                        rk2 = rqp.tile([1, N], F32, tag="rq")
                        for qt in range(4):
                            fsl = slice(qt * QC, (qt + 1) * QC)
                            ssk2 = ssqp.tile([1, QC], F32, tag="ssk")
                            nc.tensor.matmul(
                                ssk2, onesk[64:P, :], sq[64:P, fsl],
                                start=True, stop=True,
                            )
                            nc.scalar.activation(
                                rk2[:, fsl], ssk2,
                                func=AF.Abs_reciprocal_sqrt,
                                bias=epsq_t[0:1, :], scale=1.0 / 64,
                            )
                        rbk = rbp.tile([64, N], F32, tag="rb")
                        for half in range(2):
                            fsl = slice(half * N // 2, (half + 1) * N // 2)
                            nc.gpsimd.partition_broadcast(
                                rbk[:, fsl], rk2[:, fsl], channels=64
                            )
                        nc.vector.tensor_tensor(
                            out=ksb[h], in0=ksb[h], in1=rbk,
                            op=mybir.AluOpType.mult,
                        )
                        nc.vector.tensor_scalar(
                            ksb[h], ksb[h],
                            gb_sb[:, 3 + h : 4 + h], gb_sb[:, 9 + h : 10 + h],
                            op0=mybir.AluOpType.mult,
                            op1=mybir.AluOpType.add,
                        )

                    # v batches fill the PE while the stat chains drain
                    _emit_v_batch()
                    _emit_v_batch()
                while vb_count[0] < NT // 2:
                    _emit_v_batch()
                # one PE transpose per head turns the [16, 128] rstd_k
                # reshape into the [128, 16] exp-scale column form
                for h in range(HL):
                    rkT = vp.tile([P, NT], F32, tag="rkT")
                    nc.tensor.transpose(rkT, rk16s[h], ident16)
                    nc.scalar.copy(rk_cols[:, h, :], rkT)
                    # bit-trick exp scale for the DVE half: A = 128*log2(e)*s
                    nc.vector.tensor_scalar_mul(
                        rkA_cols[:, h, :], rkT, 184.6650559
                    )

            # ================= attention =================
            # q is processed in two 1024-column halves: the AV accumulator
            # shrinks to 2 PSUM banks, freeing room for THREE score buffers
            # (breaks the exp -> score-matmul -> exp WAR serialization), and
            # ACT (real exp) and DVE (bit-trick exp) alternate kt steps.
            with tc.tile_pool(name="scps", bufs=3, space="PSUM") as psc, \
                 tc.tile_pool(name="avps", bufs=1, space="PSUM") as pav, \
                 tc.tile_pool(name="expsb", bufs=3) as pexp, \
                 tc.tile_pool(name="avfsb", bufs=2) as pavf, \
                 tc.tile_pool(name="rrsb", bufs=2) as prr, \
                 tc.tile_pool(name="rbnsb", bufs=2) as prbn, \
                 tc.tile_pool(name="o1sb", bufs=1) as po1:

                for h in range(HL):
                    kT, qT = ksb[h], qhat[h]
                    escale = rk_cols[:, h, :]
                    escaleA = rkA_cols[:, h, :]
                    for half in range(2):
                        fsl = slice(half * 1024, (half + 1) * 1024)
                        av_ps = pav.tile([65, 1024], F32, tag="av")
                        for kt in range(NT):
                            sct = psc.tile([P, 1024], F32, tag="sc")
                            for q2 in range(2):
                                qsl = slice(half * 1024 + q2 * QC,
                                            half * 1024 + (q2 + 1) * QC)
                                nc.tensor.matmul(
                                    sct[:, q2 * QC : (q2 + 1) * QC],
                                    kT[:, kt * P : (kt + 1) * P], qT[:, qsl],
                                    start=True, stop=True,
                                )
                            eT = pexp.tile([P, 1024], BF16, tag="expT")
                            if kt % 2 == 0:
                                nc.scalar.activation(
                                    eT, sct, func=AF.Exp,
                                    bias=shift_t,
                                    scale=(escale[:, kt : kt + 1]
                                           if not apply_gb else SCALE),
                                )
                            else:
                                # DVE bit-trick exp: the int16 convert of
                                # A*s + B IS the bf16 pattern of
                                # exp(scale*s + shift) (~1.8% rms; verified
                                # end-to-end well inside the error budget)
                                nc.vector.tensor_scalar(
                                    eT.bitcast(I16), sct,
                                    (escaleA[:, kt : kt + 1]
                                     if not apply_gb else 23.0831),
                                    15510.336,
                                    op0=mybir.AluOpType.mult,
                                    op1=mybir.AluOpType.add,
                                )
                            for qc in range(2):
                                nc.tensor.matmul(
                                    av_ps[:, qc * QC : (qc + 1) * QC],
                                    v_all[:, kt, h, 0:65],
                                    eT[:, qc * QC : (qc + 1) * QC],
                                    start=(kt == 0), stop=(kt == NT - 1),
                                )

                        # normalize this half; overlaps the next kt loop
                        avf = pavf.tile([65, 1024], F32, tag="avf")
                        nc.vector.tensor_copy(avf, av_ps)
                        s2 = prr.tile([2, QC], F32, tag="s4")
                        nc.gpsimd.dma_start(out=s2, in_=avf[64:65, :])
                        r2 = prr.tile([2, QC], F32, tag="r4")
                        nc.vector.reciprocal_approx_fast(out=r2, in_=s2)
                        if h == 2:
                            # keep-warm fillers gated on the recip tick
                            # along the tail chain (in-order PE)
                            echo = prr.tile([2, QC], BF16, tag="echo")
                            nc.vector.tensor_copy(echo, r2)
                            for j in range(12 if half == 1 else 4):
                                sct = psc.tile([P, 1024], F32, tag="sc")
                                if j == 0:
                                    nc.tensor.matmul(
                                        sct[:, 0:QC], echo[:, 0:P],
                                        echo[:, 0:QC],
                                        start=True, stop=True,
                                    )
                                else:
                                    nc.tensor.matmul(
                                        sct[:, 0:QC],
                                        xt_sb[:, 0, 0:P], xt_sb[:, 0, 0:QC],
                                        start=True, stop=True,
                                    )
                        rr = prr.tile([1, 1024], F32, tag="rr")
                        nc.gpsimd.dma_start(out=rr, in_=r2)
                        rbn = prbn.tile([64, 1024], F32, tag="rbn")
                        nc.gpsimd.partition_broadcast(rbn, rr, channels=64)
                        if h == 0:
                            nc.vector.tensor_tensor(
                                out=oTA[0:64, fsl], in0=avf[0:64, :],
                                in1=rbn, op=mybir.AluOpType.mult,
                            )
                        elif h == 1:
                            # DVE cannot shift partitions; write base-0
                            # tmp then DMA into oTA rows 64-127
                            tmp = po1.tile([64, 1024], BF16, tag="o1tmp")
                            nc.vector.tensor_tensor(
                                out=tmp, in0=avf[0:64, :], in1=rbn,
                                op=mybir.AluOpType.mult,
                            )
                            nc.sync.dma_start(out=oTA[64:P, fsl], in_=tmp)
                        else:
                            nc.vector.tensor_tensor(
                                out=oTB[:, fsl], in0=avf[0:64, :],
                                in1=rbn, op=mybir.AluOpType.mult,
                            )

            # ================= projection =================
            with tc.tile_pool(name="pjps", bufs=3, space="PSUM") as ppj, \
                 tc.tile_pool(name="pjw", bufs=2, space="PSUM") as ppw, \
                 tc.tile_pool(name="ysb", bufs=3) as py:
                for mt in range(NT):
                    msl = slice(mt * P, (mt + 1) * P)
                    y_ps = ppj.tile([P, C], F32, tag="y")
                    # warm filler keeps the HAM activity monitor fed so the
                    # PE stays at full clock through the projection
                    warmp = ppw.tile([P, P], F32, tag="warmp")
                    nc.tensor.matmul(
                        warmp, xt_sb[:, 0, 0:P], xt_sb[:, 0, 0:P],
                        start=True, stop=True,
                    )
                    for n0, n1 in [(0, 512), (512, 768)]:
                        nc.tensor.matmul(
                            y_ps[:, n0:n1], oTA[:, msl], wpA[:, n0:n1],
                            start=True, stop=False,
                        )
                        nc.tensor.matmul(
                            y_ps[:, n0:n1], oTB[:, msl], wpB[:, n0:n1],
                            start=False, stop=True,
                        )
                    y_out = py.tile([P, C], BF16, tag="y_out")
                    # split the drain across both copy engines
                    nc.vector.tensor_copy(y_out[:, 0:384], y_ps[:, 0:384])
                    nc.scalar.copy(y_out[:, 384:768], y_ps[:, 384:768])
                    nc.sync.dma_start(out=out_d.ap()[msl, :], in_=y_out)

    nc.compile()
    return nc


_CACHED = {}


def _get_nc(apply_gb):
    key = ("nc", apply_gb)
    if key not in _CACHED:
        nc = bacc.Bacc("TRN2", target_bir_lowering=False, debug=False)
        _CACHED[key] = _build(nc, apply_gb)
    return _CACHED[key]


def _make_in_maps(inputs):
    x = np.asarray(inputs["x"], np.float32)
    wqkv = np.asarray(inputs["W_qkv"], np.float32)
    wproj = np.asarray(inputs["W_proj"], np.float32)
    qg = np.asarray(inputs["q_gamma"], np.float32)
    qb = np.asarray(inputs["q_beta"], np.float32)
    kg = np.asarray(inputs["k_gamma"], np.float32)
    kb = np.asarray(inputs["k_beta"], np.float32)

    bf = ml_dtypes.bfloat16
    w3 = wqkv.reshape(C, 3, H, D)
    cmat = np.eye(D, dtype=np.float32) - np.full((D, D), 1.0 / D, np.float32)
    in_maps = []
    for c in range(8):
        b = c // 4
        h0 = (c % 4) * HL
        cols = []
        for hh in range(HL):
            cols.append(w3[:, 0, h0 + hh, :] @ cmat)  # centered Wq
            cols.append(w3[:, 1, h0 + hh, :] @ cmat)  # centered Wk
        wqk = np.concatenate(cols, axis=1)  # [C, 384]
        wv = np.ascontiguousarray(
            w3[:, 2, h0 : h0 + HL, :].reshape(C, HL * D)
        )
        gbm = np.zeros((D, 12), np.float32)
        gbm[:, 0:3] = qg[:, None]
        gbm[:, 3:6] = kg[:, None]
        gbm[:, 6:9] = qb[:, None]
        gbm[:, 9:12] = kb[:, None]
        in_maps.append(
            {
                "xt": np.ascontiguousarray(x[b].T).astype(bf),
                "wqk": np.ascontiguousarray(wqk).astype(bf),
                "wv": wv.astype(bf),
                "wp": np.ascontiguousarray(
                    wproj[h0 * D : (h0 + HL) * D, :]
                ).astype(bf),
                "gb": gbm,
            }
        )
    return in_maps


def _gather(inputs, results):
    bproj = np.asarray(inputs["b_proj"], np.float32)
    y = np.zeros((B, N, C), np.float32)
    for c in range(8):
        y[c // 4] += np.asarray(results[c]["out"], dtype=np.float32)
    y += bproj
    return y


def _install_profile_hook():
    """The agent image's antenv lacks axon_hooks; synthesize it so
    run_bass_kernel_spmd(trace=True) can NTFF-profile via ctypes."""
    import types

    if "antenv.axon_hooks" in sys.modules:
        return
    try:
        from trn_agent_boot.trn_boot import _ntff_profile_via_ctypes

        hook = _ntff_profile_via_ctypes("/opt/axon/libaxon_pjrt.so")
    except Exception:
        hook = None
    mod = types.ModuleType("antenv.axon_hooks")
    mod.get_axon_ntff_profile_hook = lambda: hook
    mod.set_axon_ntff_profile_hook = lambda h: None
    sys.modules["antenv.axon_hooks"] = mod
    # no S3 in this container: keep artifacts local
    bass_utils.upload_artifacts = lambda tmpdir: tmpdir


def _kernel_impl(inputs, trace=False, tmpdir=None):
    apply_gb = not (
        np.all(np.asarray(inputs["q_gamma"]) == 1.0)
        and np.all(np.asarray(inputs["k_gamma"]) == 1.0)
        and np.all(np.asarray(inputs["q_beta"]) == 0.0)
        and np.all(np.asarray(inputs["k_beta"]) == 0.0)
    )
    nc = _get_nc(apply_gb)
    in_maps = _make_in_maps(inputs)
    if trace:
        _install_profile_hook()
    res = bass_utils.run_bass_kernel_spmd(
        nc, in_maps, core_ids=list(range(8)), trace=trace, tmpdir=tmpdir
    )
    out = _gather(inputs, res.results)
    return out, res


def kernel(**inputs):
    out, _ = _kernel_impl(inputs)
    return out


def kernel_with_profile(**inputs):
    out, res = _kernel_impl(inputs, trace=True)
    return out, res
